# revision 21
# baseline (speedup 1.0000x reference)
"""Multi-head causal attention with RoPE on 8 Trainium2 NeuronCores.

Reference computation (B=2, T=2048, C=1024, H=16, Dh=64, fp32):
    qkv = x @ w_qkv + b_qkv ; split q,k,v ; RoPE(q), RoPE(k)
    attn = softmax_causal(q k^T / sqrt(Dh)) @ v ; out = attn @ w_proj + b_proj

Sharding: core c = b*4 + g handles batch b and head group g (heads 4g..4g+3).
Data-parallel over batch, tensor-parallel over heads (w_qkv column-split,
w_proj row-split).  Each core emits a partial [T, C] projection output; the
host sums the 4 per-batch partials and adds b_proj.

Per-core kernel, v2.  Heavy matmuls in bf16 (fp32 PSUM accumulation).
Design is driven by two trace findings on the v1 kernel: (a) the scalar
engine is the serial bottleneck of the attention phase (exp of all causal
scores at 1 elem/cycle/lane), and (b) the PE HAM clock gate throttled the
PE to 1.2 GHz for the whole attention phase (fp32r broadcast matmuls +
ACT table reloads for the Ln/Exp reciprocal created PE idle gaps).

  - The program is emitted as one interleaved stream: while the ACT-bound
    attention chunk pipeline of head pair 0 runs, the PE executes "filler"
    matmuls (V projection tiles, QKV+RoPE for pair 1) from the same queue;
    during pair 1's attention the fillers are the output-projection tiles
    of already-normalized spans.  The PE never idles long enough to drop
    to the throttled clock.
  - S^T tile = K_j Q^T per head; the two heads of a pair are emitted
    back-to-back with K=64 stationaries at row groups 0/64 (tile_position
    auto-derived from base partitions), so they run concurrently in the
    two halves of the PE array.
  - softmax: exp on ACT with the 1/sqrt(Dh) scale fused; causal via
    narrowing each k-tile's q-range plus one triangular -400 mask matmul
    on the diagonal 128x128 block.
  - V is augmented with a ones column so PV also emits the softmax
    denominator; 1/denom via vector.reciprocal_approx_fast (no ACT table
    switch), broadcast across partitions with a stride-0-source DMA, and
    applied by DVE as the PSUM->SBUF move of the attention tile.
  - projection: per head-pair stationary attn^T tiles vs w_proj rows,
    emitted per 512-column half so each PSUM tile is one bank.
"""

import numpy as np
import ml_dtypes

from collections import deque

import concourse.bacc as bacc
import concourse.bass as bass
import concourse.mybir as mybir
from concourse.tile import TileContext
from concourse.bass_utils import run_bass_kernel_spmd

F32 = mybir.dt.float32
BF16 = mybir.dt.bfloat16
NPBF16 = np.dtype(ml_dtypes.bfloat16)

B, T, C = 2, 2048, 1024
H, DH = 16, 64
GH = 4  # heads per core
N_CORES = 8
NCHUNK = C // 128  # 8 contraction chunks
NT = T // 128  # 16 token tiles
NSPAN = T // 512  # 4 query spans
QK_COLS = 2 * GH * DH  # 512 = q cols (256) + k cols (256)
VA = GH * (DH + 1)  # 260 = v cols augmented with ones column per head
EXP = mybir.ActivationFunctionType.Exp


def _build(qk_bias=True):
    nc = bacc.Bacc("TRN2", target_bir_lowering=False, debug=False, num_devices=N_CORES)

    xT = nc.dram_tensor("xT", [C, T], BF16, kind="ExternalInput")
    wqk = nc.dram_tensor("wqk", [C, QK_COLS], BF16, kind="ExternalInput")
    wv = nc.dram_tensor("wv", [C, VA], BF16, kind="ExternalInput")
    bqk_d = nc.dram_tensor("bqk", [1, QK_COLS], BF16, kind="ExternalInput")
    bv_d = nc.dram_tensor("bv", [1, VA], BF16, kind="ExternalInput")
    cos_d = nc.dram_tensor("cosT", [128, T], BF16, kind="ExternalInput")
    sinp_d = nc.dram_tensor("sinTp", [128, T], BF16, kind="ExternalInput")
    perm_d = nc.dram_tensor("perm", [128, 128], BF16, kind="ExternalInput")
    maskT_d = nc.dram_tensor("maskT", [128, 128], BF16, kind="ExternalInput")
    id_d = nc.dram_tensor("id128", [128, 128], BF16, kind="ExternalInput")
    wproj_d = nc.dram_tensor("wproj", [2, 128, C], BF16, kind="ExternalInput")
    out_d = nc.dram_tensor("out", [T, C], BF16, kind="ExternalOutput")

    with TileContext(nc) as tc:
        with (
            tc.tile_pool(name="pers", bufs=1) as pers,
            tc.tile_pool(name="ps_s", bufs=2, space="PSUM") as ps_s,
            tc.tile_pool(name="ps_pv", bufs=2, space="PSUM") as ps_pv,
            tc.tile_pool(name="ps_fill", bufs=2, space="PSUM") as ps_fill,
            tc.tile_pool(name="sbw", bufs=1) as sbw,
        ):
            ones = pers.tile([1, 512], BF16, tag="ones")
            nc.vector.memset(ones, 1.0)
            warm = pers.tile([1, 8], F32, tag="warm")
            # Prepay the exp ACT-table load during the DMA ramp.
            nc.scalar.activation(out=warm, in_=ones[0:1, 0:8], func=EXP, scale=0.125)

            # ---------------- input DMA (multi-queue) --------------------
            xt = []
            for kc in range(NCHUNK):
                t = pers.tile([128, T], BF16, tag="xt", bufs=NCHUNK, name=f"xt{kc}")
                eng = nc.sync if kc % 2 == 0 else nc.scalar
                eng.dma_start(out=t, in_=xT[128 * kc : 128 * (kc + 1), :])
                xt.append(t)
            wqk_t = []
            for kc in range(NCHUNK):
                t = pers.tile(
                    [128, QK_COLS], BF16, tag="wqk", bufs=NCHUNK, name=f"wqk{kc}"
                )
                nc.gpsimd.dma_start(out=t, in_=wqk[128 * kc : 128 * (kc + 1), :])
                wqk_t.append(t)
            bqk_sb = pers.tile([1, QK_COLS], BF16, tag="bqk")
            nc.sync.dma_start(out=bqk_sb, in_=bqk_d[:, :])
            bv_sb = pers.tile([1, VA], BF16, tag="bv")
            nc.sync.dma_start(out=bv_sb, in_=bv_d[:, :])
            perm_sb = pers.tile([128, 128], BF16, tag="perm")
            nc.sync.dma_start(out=perm_sb, in_=perm_d[:, :])
            mask_sb = pers.tile([128, 128], BF16, tag="maskT")
            nc.sync.dma_start(out=mask_sb, in_=maskT_d[:, :])
            id_sb = pers.tile([128, 128], BF16, tag="id128")
            nc.sync.dma_start(out=id_sb, in_=id_d[:, :])
            wv_t = []
            for kc in range(NCHUNK):
                t = pers.tile([128, VA], BF16, tag="wv", bufs=NCHUNK, name=f"wv{kc}")
                nc.gpsimd.dma_start(out=t, in_=wv[128 * kc : 128 * (kc + 1), :])
                wv_t.append(t)
            cos_sb = pers.tile([128, T], BF16, tag="cos")
            nc.gpsimd.dma_start(out=cos_sb, in_=cos_d[:, :])
            sinp_sb = pers.tile([128, T], BF16, tag="sinp")
            nc.gpsimd.dma_start(out=sinp_sb, in_=sinp_d[:, :])
            wproj_sb = []
            for p in range(2):
                t = pers.tile([128, C], BF16, tag="wproj", bufs=2, name=f"wproj{p}")
                nc.scalar.dma_start(out=t, in_=wproj_d[p, :, :])
                wproj_sb.append(t)

            # Persistent intermediate tiles
            qkt = []  # 4 tiles [128, T]: Q heads(0,1), Q(2,3), K(0,1), K(2,3)
            for i in range(4):
                t = pers.tile([128, T], BF16, tag="qkt", bufs=4, name=f"qkt{i}")
                qkt.append(t)
            vaug = []  # 16 tiles [128, VA], k-tile-major natural layout V
            for j in range(NT):
                t = pers.tile([128, VA], BF16, tag="vaug", bufs=NT, name=f"vaug{j}")
                vaug.append(t)
            attn = []  # 2 tiles [128, T]: normalized attn^T for head pairs
            for p in range(2):
                t = pers.tile([128, T], BF16, tag="attn", bufs=2, name=f"attn{p}")
                attn.append(t)

            # ---------------- emission helpers ---------------------------
            def rope(ct, sp, pq, permpool, permtag):
                # qkt[ct][:, ss] = pq*cos + perm @ (pq*sin_perm)
                ss = slice(512 * sp, 512 * (sp + 1))
                t2 = sbw.tile([128, 512], BF16, tag="t2", bufs=3, name="t2")
                nc.vector.tensor_mul(t2, pq, sinp_sb[:, ss])
                pp = permpool.tile([128, 512], F32, tag=permtag, name="psperm")
                nc.tensor.matmul(pp, perm_sb, t2, start=True, stop=True)
                nc.vector.tensor_mul(qkt[ct][:, ss], pq, cos_sb[:, ss])
                nc.vector.tensor_add(qkt[ct][:, ss], qkt[ct][:, ss], pp)

            def dummy(n):
                # cheap N=64 matmuls that keep the PE pipeline streaming
                # through DMA waits (the PE clock needs ~3us of unbroken
                # execution to reach 2.4 GHz; any stall resets it)
                dt = ps_pv.tile([1, 64], F32, tag="pv", name="psdum")
                for _ in range(n):
                    nc.tensor.matmul(
                        dt, ones[0:1, 0:1], ones[0:1, 0:64], start=True, stop=True
                    )

            def qk_bias_mm(tile, cs):
                if qk_bias:
                    nc.tensor.matmul(
                        tile, bqk_sb[0:1, cs], ones, start=False, stop=True
                    )

            def phase_a():
                # K (ct=2, all 4 spans) and Q spans 0-1 (ct=0) kc-outer,
                # chasing the x DMA with small dummy matmuls bridging the
                # chunk-arrival gaps.  Q spans 2-3 go to the backlog.
                cs2 = slice(256, 384)
                cs0 = slice(0, 128)
                big2 = [
                    ps_s.tile([128, 1024], F32, tag="s", name="psqk2")
                    for _ in range(2)
                ]
                t2of = lambda sp: big2[sp // 2][
                    :, 512 * (sp % 2) : 512 * (sp % 2) + 512
                ]
                t01 = [
                    ps_fill.tile([128, 512], F32, tag="ps_fill", name="psqk0")
                    for _ in range(2)
                ]
                laststop = (not qk_bias)
                for kc in range(NCHUNK):
                    st = laststop and kc == NCHUNK - 1
                    for sp in range(NSPAN):
                        nc.tensor.matmul(
                            t2of(sp),
                            wqk_t[kc][:, cs2],
                            xt[kc][:, 512 * sp : 512 * (sp + 1)],
                            start=(kc == 0),
                            stop=st,
                        )
                    for sp in range(2):
                        nc.tensor.matmul(
                            t01[sp],
                            wqk_t[kc][:, cs0],
                            xt[kc][:, 512 * sp : 512 * (sp + 1)],
                            start=(kc == 0),
                            stop=st,
                        )
                    dummy(8)
                for sp in range(2):
                    qk_bias_mm(t01[sp], cs0)
                    rope(0, sp, t01[sp], ps_fill, "ps_fill")
                for sp in range(NSPAN):
                    qk_bias_mm(t2of(sp), cs2)
                    rope(2, sp, t2of(sp), ps_fill, "ps_fill")

            def q_sp23(chunked):
                # Q spans 2-3 (ct=0), emitted as backlog right after phase A
                cs0 = slice(0, 128)
                t23 = [
                    ps_fill.tile([128, 512], F32, tag="ps_fill", name="psqk0b")
                    for _ in range(2)
                ]
                for kc in range(NCHUNK):
                    for i, sp in enumerate((2, 3)):
                        nc.tensor.matmul(
                            t23[i],
                            wqk_t[kc][:, cs0],
                            xt[kc][:, 512 * sp : 512 * (sp + 1)],
                            start=(kc == 0),
                            stop=(not qk_bias) and kc == NCHUNK - 1,
                        )
                    if chunked:
                        yield
                for i, sp in enumerate((2, 3)):
                    qk_bias_mm(t23[i], cs0)
                    rope(0, sp, t23[i], ps_fill, "ps_fill")
                    if chunked:
                        yield

            def qk_block(ct, qkpool, qktag, chunked):
                # filler variant: kc-inner per span-halfpair, fine yields
                cs = slice(128 * ct, 128 * (ct + 1))
                for half in range(2):
                    sps = (2 * half, 2 * half + 1)
                    tiles = {}
                    for sp in sps:
                        tiles[sp] = qkpool.tile(
                            [128, 512], F32, tag=qktag, name="psqk"
                        )
                    for kc in range(NCHUNK):
                        for sp in sps:
                            nc.tensor.matmul(
                                tiles[sp],
                                wqk_t[kc][:, cs],
                                xt[kc][:, 512 * sp : 512 * (sp + 1)],
                                start=(kc == 0),
                                stop=(not qk_bias) and kc == NCHUNK - 1,
                            )
                        if chunked:
                            yield
                    for sp in sps:
                        if qk_bias:
                            nc.tensor.matmul(
                                tiles[sp],
                                bqk_sb[0:1, cs],
                                ones,
                                start=False,
                                stop=True,
                            )
                        rope(ct, sp, tiles[sp], ps_fill, "ps_fill")
                        if chunked:
                            yield

            def v_tile(it, chunked):
                pv = ps_fill.tile([128, VA], F32, tag="ps_fill", name="psv")
                ts = slice(128 * it, 128 * (it + 1))
                for kc in range(NCHUNK):
                    nc.tensor.matmul(
                        pv, xt[kc][:, ts], wv_t[kc], start=(kc == 0), stop=False
                    )
                    if chunked and kc % 2 == 1 and kc < 7:
                        yield
                nc.tensor.matmul(pv, ones[0:1, 0:128], bv_sb, start=False, stop=True)
                nc.vector.tensor_copy(vaug[it], pv)
                if chunked:
                    yield

            def normalize(pair, idx, s, pv):
                # attn = pv[0:64] * (1/colsum).  The denominator (ones-column
                # PV output, PSUM row 64) is copied to partition 0 — the
                # custom-DVE reciprocal only works at base partition 0 — then
                # broadcast across partitions on GPSIMD.  No ACT tables, no
                # fp32r matmuls (both throttled the v1 kernel).
                po = idx * 64
                ss = slice(512 * s, 512 * (s + 1))
                d0 = sbw.tile([1, 512], F32, tag="d0", bufs=2, name="d0")
                nc.vector.tensor_copy(d0, pv[64:65, :])
                r = sbw.tile([1, 512], F32, tag="r", bufs=2, name="r")
                nc.vector.reciprocal_approx_fast(out=r, in_=d0)
                rbs = sbw.tile([64, 512], F32, tag="rbs", bufs=3, name="rbs")
                nc.gpsimd.partition_broadcast(rbs, r)
                nc.vector.tensor_mul(attn[pair][po : po + 64, ss], pv[0:64, :], rbs)

            def proj_half(it, nh):
                # out[ts, ns] = sum_p attn[p][:, ts]^T @ wproj[p][:, ns]
                ts = slice(128 * it, 128 * (it + 1))
                ns = slice(512 * nh, 512 * (nh + 1))
                ppj = ps_fill.tile([128, 512], F32, tag="ps_fill", name="psproj")
                for p in range(2):
                    nc.tensor.matmul(
                        ppj,
                        attn[p][:, ts],
                        wproj_sb[p][:, ns],
                        start=(p == 0),
                        stop=(p == 1),
                    )
                ob = sbw.tile([128, 512], BF16, tag="ob", bufs=4, name="ob")
                nc.vector.tensor_copy(ob, ppj)
                eng = nc.sync if (it + nh) % 2 == 0 else nc.scalar
                eng.dma_start(out=out_d[ts, ns], in_=ob)

            # ---------------- phase A: dense PE ramp ---------------------
            dummy(8)
            phase_a()

            # ------------- two-phase attention with a PE backlog ---------
            # pass1(pair, s): S + fused exp per k-tile, et tiles -> SBUF.
            # pass2(pair, s): PV + normalize, emitted later as backlog
            # thunks between pass1 steps so the PE always has dense,
            # ACT-independent work (the PE only reaches 2.4 GHz after ~3us
            # of continuous execution; any stall resets it to 1.2 GHz).
            backlog = deque()

            def emit_budget(budget):
                while budget > 0 and backlog:
                    cost, fn = backlog.popleft()
                    fn()
                    budget -= cost
                return budget

            def gen_thunks(gen, n, cost):
                return [(cost, (lambda g=gen: next(g, None))) for _ in range(n)]

            kq1_done = [False]

            def mark_kq1():
                kq1_done[0] = True

            backlog.extend(gen_thunks(q_sp23(True), 10, 1024))
            backlog.extend(
                th for it in range(NT) for th in gen_thunks(v_tile(it, True), 4, 550)
            )
            backlog.extend(gen_thunks(qk_block(3, ps_fill, "ps_fill", True), 20, 1024))
            backlog.extend(gen_thunks(qk_block(1, ps_fill, "ps_fill", True), 20, 1024))
            backlog.append((0, mark_kq1))

            def pass1(pair, s):
                qt, kt = qkt[pair], qkt[2 + pair]
                cells = []
                for j in range(4 * s + 4):
                    st = ps_s.tile([128, 1024], F32, tag="s", name="st")
                    q0 = max(512 * s, 128 * j)
                    w = 512 * (s + 1) - q0
                    diag = s == j // 4
                    for idx in (0, 1):
                        po = idx * 64
                        nc.tensor.matmul(
                            st[:, 512 * idx : 512 * idx + w],
                            kt[po : po + 64, 128 * j : 128 * (j + 1)],
                            qt[po : po + 64, q0 : q0 + w],
                            start=True,
                            stop=not diag,
                        )
                    if diag:
                        for idx in (0, 1):
                            nc.tensor.matmul(
                                st[:, 512 * idx : 512 * idx + 128],
                                mask_sb,
                                id_sb,
                                start=False,
                                stop=True,
                            )
                    et = sbw.tile([128, 1024], BF16, tag="et", bufs=36, name="et")
                    if w == 512:
                        nc.scalar.activation(
                            out=et[:, :], in_=st[:, :], func=EXP, scale=0.125
                        )
                    else:
                        iv = st[:, :].rearrange("p (h c) -> p h c", h=2)[:, :, 0:w]
                        ov = et[:, :].rearrange("p (h c) -> p h c", h=2)[:, :, 0:w]
                        nc.scalar.activation(out=ov, in_=iv, func=EXP, scale=0.125)
                    cells.append((j, q0, w, et))
                    emit_budget(1000 + 3 * w)
                return cells

            def make_pass2(pair, s, cells):
                heads = (2 * pair, 2 * pair + 1)
                hold = {}
                ths = []
                for i, (j, q0, w, et) in enumerate(cells):
                    def th(i=i, j=j, q0=q0, w=w, et=et):
                        if i == 0:
                            hold["pv"] = [
                                ps_pv.tile([65, 512], F32, tag="pv", name=f"pspv{k}")
                                for k in (0, 1)
                            ]
                        for idx in (0, 1):
                            h = heads[idx]
                            nc.tensor.matmul(
                                hold["pv"][idx][:, q0 - 512 * s :],
                                vaug[j][:, 65 * h : 65 * (h + 1)],
                                et[:, 512 * idx : 512 * idx + w],
                                start=(j == 0),
                                stop=(j == 4 * s + 3),
                            )
                    ths.append((2 * w, th))

                def fin():
                    for idx in (0, 1):
                        normalize(pair, idx, s, hold["pv"][idx])
                    if pair == 1:
                        # proj right behind the normalize it depends on, so
                        # the PE has work while the normalize chain runs
                        pr = [
                            (1300, (lambda it=it, nh=nh: proj_half(it, nh)))
                            for it in range(4 * s, 4 * s + 4)
                            for nh in range(2)
                        ]
                        backlog.extendleft(reversed(pr))

                ths.append((400, fin))
                return ths

            for s in (0, 1, 2, 3):
                cells = pass1(0, s)
                backlog.extend(make_pass2(0, s, cells))
            while not kq1_done[0]:
                emit_budget(1)
            for s in (3, 2, 1, 0):
                cells = pass1(1, s)
                backlog.extend(make_pass2(1, s, cells))
            while backlog:
                emit_budget(1)

    nc.compile()
    return nc


_NC = {}


def _get_nc(qk_bias=True):
    if qk_bias not in _NC:
        _NC[qk_bias] = _build(qk_bias=qk_bias)
    return _NC[qk_bias]


def _rope_tables():
    theta = (10000.0 ** (-np.arange(0, DH, 2, dtype=np.float32) / DH)).astype(
        np.float32
    )
    t = np.arange(T, dtype=np.float32)
    sinusoid = np.outer(t, theta).astype(np.float32)  # [T, DH/2]
    sin = np.concatenate([np.sin(sinusoid), np.sin(sinusoid)], axis=1)  # [T, DH]
    cos = np.concatenate([np.cos(sinusoid), np.cos(sinusoid)], axis=1)
    cosT = cos.T  # [DH, T]
    sinT = sin.T
    # sin_perm[e] = sin[(e+32) % 64]
    idx = (np.arange(DH) + 32) % DH
    sinTp = sinT[idx]
    cos2 = np.ascontiguousarray(np.concatenate([cosT, cosT], axis=0))  # [128, T]
    sinp2 = np.ascontiguousarray(np.concatenate([sinTp, sinTp], axis=0))
    return _bf(cos2), _bf(sinp2)


def _perm_matrix():
    p = np.zeros((128, 128), dtype=np.float32)
    for m in range(128):
        blk = m // 64
        k = blk * 64 + (m % 64 + 32) % 64
        p[k, m] = 1.0
    return p


def _mask_matrices():
    # maskT.T @ I adds -400 to S^T[k, q] where k > q (then exp(0.125*s)=0):
    # maskT[a, b] = -400 where b > a
    maskT = -400.0 * np.triu(np.ones((128, 128), dtype=np.float32), 1)
    return maskT, np.eye(128, dtype=np.float32)


def _bf(a):
    return np.ascontiguousarray(np.asarray(a, dtype=np.float32).astype(NPBF16))


def _prepare_in_maps(x, w_qkv, b_qkv, w_proj):
    x = np.asarray(x, dtype=np.float32)
    w_qkv = np.asarray(w_qkv, dtype=np.float32)
    b_qkv = np.asarray(b_qkv, dtype=np.float32)
    w_proj = np.asarray(w_proj, dtype=np.float32)

    cos2, sinp2 = _rope_tables()
    perm = _bf(_perm_matrix())
    maskT, id128 = _mask_matrices()
    maskT, id128 = _bf(maskT), _bf(id128)
    xTs = [_bf(x[b].T) for b in range(B)]

    in_maps = []
    for c in range(N_CORES):
        b, g = divmod(c, 4)
        h0 = g * GH  # first head of the group
        qcols = w_qkv[:, h0 * DH : (h0 + GH) * DH]
        kcols = w_qkv[:, C + h0 * DH : C + (h0 + GH) * DH]
        wqk = _bf(np.concatenate([qcols, kcols], axis=1))
        wv = np.zeros((C, VA), dtype=np.float32)
        bv = np.zeros((1, VA), dtype=np.float32)
        for j in range(GH):
            src = 2 * C + (h0 + j) * DH
            wv[:, j * 65 : j * 65 + DH] = w_qkv[:, src : src + DH]
            bv[0, j * 65 : j * 65 + DH] = b_qkv[src : src + DH]
            bv[0, j * 65 + DH] = 1.0
        bqk = np.concatenate(
            [b_qkv[h0 * DH : (h0 + GH) * DH], b_qkv[C + h0 * DH : C + (h0 + GH) * DH]]
        ).reshape(1, QK_COLS)
        wproj = np.stack(
            [w_proj[(h0 + 2 * p) * DH : (h0 + 2 * p + 2) * DH, :] for p in range(2)]
        )
        in_maps.append(
            {
                "xT": xTs[b],
                "wqk": wqk,
                "wv": _bf(wv),
                "bqk": _bf(bqk),
                "bv": _bf(bv),
                "cosT": cos2,
                "sinTp": sinp2,
                "perm": perm,
                "maskT": maskT,
                "id128": id128,
                "wproj": _bf(wproj),
            }
        )
    return in_maps


def run(x, w_qkv, b_qkv, w_proj, b_proj, trace=False, tmpdir=None):
    qk_bias = bool(np.any(np.asarray(b_qkv, dtype=np.float32)[: 2 * C]))
    nc = _get_nc(qk_bias)
    in_maps = _prepare_in_maps(x, w_qkv, b_qkv, w_proj)
    res = run_bass_kernel_spmd(
        nc, in_maps, list(range(N_CORES)), trace=trace, tmpdir=tmpdir
    )
    b_proj = np.asarray(b_proj, dtype=np.float32)
    out = np.empty((B, T, C), dtype=np.float32)
    for b in range(B):
        acc = res.results[4 * b]["out"].astype(np.float32)
        for g in range(1, 4):
            acc = acc + res.results[4 * b + g]["out"].astype(np.float32)
        out[b] = acc + b_proj
    return out, res


def kernel(x, w_qkv, b_qkv, w_proj, b_proj):
    out, _ = run(x, w_qkv, b_qkv, w_proj, b_proj, trace=False)
    return out


# revision 23
# speedup vs baseline: 1.1378x; 1.1378x over previous
"""Multi-head causal attention with RoPE on 8 Trainium2 NeuronCores.

Reference computation (B=2, T=2048, C=1024, H=16, Dh=64, fp32):
    qkv = x @ w_qkv + b_qkv ; split q,k,v ; RoPE(q), RoPE(k)
    attn = softmax_causal(q k^T / sqrt(Dh)) @ v ; out = attn @ w_proj + b_proj

Sharding: core c = b*4 + g handles batch b and head group g (heads 4g..4g+3).
Data-parallel over batch, tensor-parallel over heads (w_qkv column-split,
w_proj row-split).  Each core emits a partial [T, C] projection output; the
host sums the 4 per-batch partials and adds b_proj.

Per-core kernel, v2.  Heavy matmuls in bf16 (fp32 PSUM accumulation).
Design is driven by two trace findings on the v1 kernel: (a) the scalar
engine is the serial bottleneck of the attention phase (exp of all causal
scores at 1 elem/cycle/lane), and (b) the PE HAM clock gate throttled the
PE to 1.2 GHz for the whole attention phase (fp32r broadcast matmuls +
ACT table reloads for the Ln/Exp reciprocal created PE idle gaps).

  - The program is emitted as one interleaved stream: while the ACT-bound
    attention chunk pipeline of head pair 0 runs, the PE executes "filler"
    matmuls (V projection tiles, QKV+RoPE for pair 1) from the same queue;
    during pair 1's attention the fillers are the output-projection tiles
    of already-normalized spans.  The PE never idles long enough to drop
    to the throttled clock.
  - S^T tile = K_j Q^T per head; the two heads of a pair are emitted
    back-to-back with K=64 stationaries at row groups 0/64 (tile_position
    auto-derived from base partitions), so they run concurrently in the
    two halves of the PE array.
  - softmax: exp on ACT with the 1/sqrt(Dh) scale fused; causal via
    narrowing each k-tile's q-range plus one triangular -400 mask matmul
    on the diagonal 128x128 block.
  - V is augmented with a ones column so PV also emits the softmax
    denominator; 1/denom via vector.reciprocal_approx_fast (no ACT table
    switch), broadcast across partitions with a stride-0-source DMA, and
    applied by DVE as the PSUM->SBUF move of the attention tile.
  - projection: per head-pair stationary attn^T tiles vs w_proj rows,
    emitted per 512-column half so each PSUM tile is one bank.
"""

import numpy as np
import ml_dtypes

from collections import deque

import concourse.bacc as bacc
import concourse.bass as bass
import concourse.mybir as mybir
from concourse.tile import TileContext
from concourse.bass_utils import run_bass_kernel_spmd

F32 = mybir.dt.float32
BF16 = mybir.dt.bfloat16
NPBF16 = np.dtype(ml_dtypes.bfloat16)

B, T, C = 2, 2048, 1024
H, DH = 16, 64
GH = 4  # heads per core
N_CORES = 8
NCHUNK = C // 128  # 8 contraction chunks
NT = T // 128  # 16 token tiles
NSPAN = T // 512  # 4 query spans
QK_COLS = 2 * GH * DH  # 512 = q cols (256) + k cols (256)
VA = GH * (DH + 1)  # 260 = v cols augmented with ones column per head
EXP = mybir.ActivationFunctionType.Exp


def _build(qk_bias=True):
    nc = bacc.Bacc("TRN2", target_bir_lowering=False, debug=False, num_devices=N_CORES)

    xT = nc.dram_tensor("xT", [C, T], BF16, kind="ExternalInput")
    wqk = nc.dram_tensor("wqk", [C, QK_COLS], BF16, kind="ExternalInput")
    wv = nc.dram_tensor("wv", [C, VA], BF16, kind="ExternalInput")
    bqk_d = nc.dram_tensor("bqk", [1, QK_COLS], BF16, kind="ExternalInput")
    bv_d = nc.dram_tensor("bv", [1, VA], BF16, kind="ExternalInput")
    cos_d = nc.dram_tensor("cosT", [128, T], BF16, kind="ExternalInput")
    sinp_d = nc.dram_tensor("sinTp", [128, T], BF16, kind="ExternalInput")
    perm_d = nc.dram_tensor("perm", [128, 128], BF16, kind="ExternalInput")
    maskT_d = nc.dram_tensor("maskT", [128, 128], BF16, kind="ExternalInput")
    id_d = nc.dram_tensor("id128", [128, 128], BF16, kind="ExternalInput")
    wproj_d = nc.dram_tensor("wproj", [2, 128, C], BF16, kind="ExternalInput")
    out_d = nc.dram_tensor("out", [T, C], BF16, kind="ExternalOutput")

    with TileContext(nc) as tc:
        with (
            tc.tile_pool(name="pers", bufs=1) as pers,
            tc.tile_pool(name="ps_s", bufs=2, space="PSUM") as ps_s,
            tc.tile_pool(name="ps_pv", bufs=2, space="PSUM") as ps_pv,
            tc.tile_pool(name="ps_fill", bufs=2, space="PSUM") as ps_fill,
            tc.tile_pool(name="sbw", bufs=1) as sbw,
        ):
            ones = pers.tile([1, 512], BF16, tag="ones")
            nc.vector.memset(ones, 1.0)
            warm = pers.tile([1, 8], F32, tag="warm")
            # Prepay the exp ACT-table load during the DMA ramp.
            nc.scalar.activation(out=warm, in_=ones[0:1, 0:8], func=EXP, scale=0.125)

            # ---------------- input DMA (multi-queue) --------------------
            xt = []
            for kc in range(NCHUNK):
                t = pers.tile([128, T], BF16, tag="xt", bufs=NCHUNK, name=f"xt{kc}")
                eng = nc.sync if kc % 2 == 0 else nc.scalar
                eng.dma_start(out=t, in_=xT[128 * kc : 128 * (kc + 1), :])
                xt.append(t)
            wqk_t = []
            for kc in range(NCHUNK):
                t = pers.tile(
                    [128, QK_COLS], BF16, tag="wqk", bufs=NCHUNK, name=f"wqk{kc}"
                )
                nc.gpsimd.dma_start(out=t, in_=wqk[128 * kc : 128 * (kc + 1), :])
                wqk_t.append(t)
            bqk_sb = pers.tile([1, QK_COLS], BF16, tag="bqk")
            nc.sync.dma_start(out=bqk_sb, in_=bqk_d[:, :])
            bv_sb = pers.tile([1, VA], BF16, tag="bv")
            nc.sync.dma_start(out=bv_sb, in_=bv_d[:, :])
            perm_sb = pers.tile([128, 128], BF16, tag="perm")
            nc.sync.dma_start(out=perm_sb, in_=perm_d[:, :])
            mask_sb = pers.tile([128, 128], BF16, tag="maskT")
            nc.sync.dma_start(out=mask_sb, in_=maskT_d[:, :])
            id_sb = pers.tile([128, 128], BF16, tag="id128")
            nc.sync.dma_start(out=id_sb, in_=id_d[:, :])
            wv_t = []
            for kc in range(NCHUNK):
                t = pers.tile([128, VA], BF16, tag="wv", bufs=NCHUNK, name=f"wv{kc}")
                nc.gpsimd.dma_start(out=t, in_=wv[128 * kc : 128 * (kc + 1), :])
                wv_t.append(t)
            cos_sb = pers.tile([128, T], BF16, tag="cos")
            nc.gpsimd.dma_start(out=cos_sb, in_=cos_d[:, :])
            sinp_sb = pers.tile([128, T], BF16, tag="sinp")
            nc.gpsimd.dma_start(out=sinp_sb, in_=sinp_d[:, :])
            wproj_sb = []
            for p in range(2):
                t = pers.tile([128, C], BF16, tag="wproj", bufs=2, name=f"wproj{p}")
                nc.scalar.dma_start(out=t, in_=wproj_d[p, :, :])
                wproj_sb.append(t)

            # Persistent intermediate tiles
            qkt = []  # 4 tiles [128, T]: Q heads(0,1), Q(2,3), K(0,1), K(2,3)
            for i in range(4):
                t = pers.tile([128, T], BF16, tag="qkt", bufs=4, name=f"qkt{i}")
                qkt.append(t)
            vaug = []  # 16 tiles [128, VA], k-tile-major natural layout V
            for j in range(NT):
                t = pers.tile([128, VA], BF16, tag="vaug", bufs=NT, name=f"vaug{j}")
                vaug.append(t)
            attn = []  # 2 tiles [128, T]: normalized attn^T for head pairs
            for p in range(2):
                t = pers.tile([128, T], BF16, tag="attn", bufs=2, name=f"attn{p}")
                attn.append(t)

            # ---------------- emission helpers ---------------------------
            def rope(ct, sp, pq, permpool, permtag):
                # qkt[ct][:, ss] = pq*cos + perm @ (pq*sin_perm)
                ss = slice(512 * sp, 512 * (sp + 1))
                t2 = sbw.tile([128, 512], BF16, tag="t2", bufs=3, name="t2")
                nc.vector.tensor_mul(t2, pq, sinp_sb[:, ss])
                pp = permpool.tile([128, 512], F32, tag=permtag, name="psperm")
                nc.tensor.matmul(pp, perm_sb, t2, start=True, stop=True)
                nc.vector.tensor_mul(qkt[ct][:, ss], pq, cos_sb[:, ss])
                nc.vector.tensor_add(qkt[ct][:, ss], qkt[ct][:, ss], pp)


            def qk_bias_mm(tile, cs):
                if qk_bias:
                    nc.tensor.matmul(
                        tile, bqk_sb[0:1, cs], ones, start=False, stop=True
                    )

            def qk_dense(ct):
                # one Q-or-K column tile [128, T], kc-outer over all 4 spans
                # (PSUM: two 2-bank ps_s tiles), chasing the x DMA.
                cs = slice(128 * ct, 128 * (ct + 1))
                big = [
                    ps_s.tile([128, 1024], F32, tag="s", name="psqk2")
                    for _ in range(2)
                ]
                tile_of = lambda sp: big[sp // 2][
                    :, 512 * (sp % 2) : 512 * (sp % 2) + 512
                ]
                for kc in range(NCHUNK):
                    for sp in range(NSPAN):
                        nc.tensor.matmul(
                            tile_of(sp),
                            wqk_t[kc][:, cs],
                            xt[kc][:, 512 * sp : 512 * (sp + 1)],
                            start=(kc == 0),
                            stop=(not qk_bias) and kc == NCHUNK - 1,
                        )
                for sp in range(NSPAN):
                    qk_bias_mm(tile_of(sp), cs)
                    rope(ct, sp, tile_of(sp), ps_fill, "ps_fill")

            def qk_block(ct, qkpool, qktag, chunked):
                # filler variant: kc-inner per span-halfpair, fine yields
                cs = slice(128 * ct, 128 * (ct + 1))
                for half in range(2):
                    sps = (2 * half, 2 * half + 1)
                    tiles = {}
                    for sp in sps:
                        tiles[sp] = qkpool.tile(
                            [128, 512], F32, tag=qktag, name="psqk"
                        )
                    for kc in range(NCHUNK):
                        for sp in sps:
                            nc.tensor.matmul(
                                tiles[sp],
                                wqk_t[kc][:, cs],
                                xt[kc][:, 512 * sp : 512 * (sp + 1)],
                                start=(kc == 0),
                                stop=(not qk_bias) and kc == NCHUNK - 1,
                            )
                        if chunked:
                            yield
                    for sp in sps:
                        if qk_bias:
                            nc.tensor.matmul(
                                tiles[sp],
                                bqk_sb[0:1, cs],
                                ones,
                                start=False,
                                stop=True,
                            )
                        rope(ct, sp, tiles[sp], ps_fill, "ps_fill")
                        if chunked:
                            yield

            def v_tile(it, chunked):
                pv = ps_fill.tile([128, VA], F32, tag="ps_fill", name="psv")
                ts = slice(128 * it, 128 * (it + 1))
                for kc in range(NCHUNK):
                    nc.tensor.matmul(
                        pv, xt[kc][:, ts], wv_t[kc], start=(kc == 0), stop=False
                    )
                    if chunked and kc % 2 == 1 and kc < 7:
                        yield
                nc.tensor.matmul(pv, ones[0:1, 0:128], bv_sb, start=False, stop=True)
                nc.vector.tensor_copy(vaug[it], pv)
                if chunked:
                    yield

            def normalize(pair, idx, s, pv):
                # attn = pv[0:64] * (1/colsum).  The denominator (ones-column
                # PV output, PSUM row 64) is copied to partition 0 — the
                # custom-DVE reciprocal only works at base partition 0 — then
                # broadcast across partitions on GPSIMD.  No ACT tables, no
                # fp32r matmuls (both throttled the v1 kernel).
                po = idx * 64
                ss = slice(512 * s, 512 * (s + 1))
                d0 = sbw.tile([1, 512], F32, tag="d0", bufs=2, name="d0")
                nc.vector.tensor_copy(d0, pv[64:65, :])
                r = sbw.tile([1, 512], F32, tag="r", bufs=2, name="r")
                nc.vector.reciprocal_approx_fast(out=r, in_=d0)
                rbs = sbw.tile([64, 512], F32, tag="rbs", bufs=3, name="rbs")
                nc.gpsimd.partition_broadcast(rbs, r)
                nc.vector.tensor_mul(attn[pair][po : po + 64, ss], pv[0:64, :], rbs)

            def proj_half(it, nh):
                # out[ts, ns] = sum_p attn[p][:, ts]^T @ wproj[p][:, ns]
                ts = slice(128 * it, 128 * (it + 1))
                ns = slice(512 * nh, 512 * (nh + 1))
                ppj = ps_fill.tile([128, 512], F32, tag="ps_fill", name="psproj")
                for p in range(2):
                    nc.tensor.matmul(
                        ppj,
                        attn[p][:, ts],
                        wproj_sb[p][:, ns],
                        start=(p == 0),
                        stop=(p == 1),
                    )
                ob = sbw.tile([128, 512], BF16, tag="ob", bufs=4, name="ob")
                if it < 8:
                    # spans 0-1 of pair 1 are projected after the last exp;
                    # the then-idle ACT takes their PSUM evacuation
                    nc.scalar.copy(ob, ppj)
                else:
                    nc.vector.tensor_copy(ob, ppj)
                eng = nc.sync if (it + nh) % 2 == 0 else nc.scalar
                eng.dma_start(out=out_d[ts, ns], in_=ob)

            # ---------------- phase A: dense PE ramp ---------------------
            qk_dense(2)
            qk_dense(0)

            # ------------- two-phase attention with a PE backlog ---------
            # pass1(pair, s): S + fused exp per k-tile, et tiles -> SBUF.
            # pass2(pair, s): PV + normalize, emitted later as backlog
            # thunks between pass1 steps so the PE always has dense,
            # ACT-independent work (the PE only reaches 2.4 GHz after ~3us
            # of continuous execution; any stall resets it to 1.2 GHz).
            backlog = deque()

            def emit_budget(budget):
                while budget > 0 and backlog:
                    cost, fn = backlog.popleft()
                    fn()
                    budget -= cost
                return budget

            def gen_thunks(gen, n, cost):
                return [(cost, (lambda g=gen: next(g, None))) for _ in range(n)]

            kq1_done = [False]

            def mark_kq1():
                kq1_done[0] = True

            backlog.extend(
                th for it in range(NT) for th in gen_thunks(v_tile(it, True), 4, 550)
            )
            backlog.extend(gen_thunks(qk_block(3, ps_fill, "ps_fill", True), 20, 1024))
            backlog.extend(gen_thunks(qk_block(1, ps_fill, "ps_fill", True), 20, 1024))
            backlog.append((0, mark_kq1))

            def pass1(pair, s):
                qt, kt = qkt[pair], qkt[2 + pair]
                cells = []
                for j in range(4 * s + 4):
                    st = ps_s.tile([128, 1024], F32, tag="s", name="st")
                    q0 = max(512 * s, 128 * j)
                    w = 512 * (s + 1) - q0
                    diag = s == j // 4
                    for idx in (0, 1):
                        po = idx * 64
                        nc.tensor.matmul(
                            st[:, 512 * idx : 512 * idx + w],
                            kt[po : po + 64, 128 * j : 128 * (j + 1)],
                            qt[po : po + 64, q0 : q0 + w],
                            start=True,
                            stop=not diag,
                        )
                    if diag:
                        for idx in (0, 1):
                            nc.tensor.matmul(
                                st[:, 512 * idx : 512 * idx + 128],
                                mask_sb,
                                id_sb,
                                start=False,
                                stop=True,
                            )
                    et = sbw.tile([128, 1024], BF16, tag="et", bufs=36, name="et")
                    if w == 512:
                        nc.scalar.activation(
                            out=et[:, :], in_=st[:, :], func=EXP, scale=0.125
                        )
                    else:
                        iv = st[:, :].rearrange("p (h c) -> p h c", h=2)[:, :, 0:w]
                        ov = et[:, :].rearrange("p (h c) -> p h c", h=2)[:, :, 0:w]
                        nc.scalar.activation(out=ov, in_=iv, func=EXP, scale=0.125)
                    cells.append((j, q0, w, et))
                    emit_budget(500 + 2 * w)
                return cells

            def make_pass2(pair, s, cells):
                heads = (2 * pair, 2 * pair + 1)
                hold = {}
                ths = []
                for i, (j, q0, w, et) in enumerate(cells):
                    def th(i=i, j=j, q0=q0, w=w, et=et):
                        if i == 0:
                            hold["pv"] = [
                                ps_pv.tile([65, 512], F32, tag="pv", name=f"pspv{k}")
                                for k in (0, 1)
                            ]
                        for idx in (0, 1):
                            h = heads[idx]
                            nc.tensor.matmul(
                                hold["pv"][idx][:, q0 - 512 * s :],
                                vaug[j][:, 65 * h : 65 * (h + 1)],
                                et[:, 512 * idx : 512 * idx + w],
                                start=(j == 0),
                                stop=(j == 4 * s + 3),
                            )
                    ths.append((2 * w, th))

                def fin():
                    for idx in (0, 1):
                        normalize(pair, idx, s, hold["pv"][idx])
                    if pair == 1:
                        # proj right behind the normalize it depends on, so
                        # the PE has work while the normalize chain runs
                        pr = [
                            (1300, (lambda it=it, nh=nh: proj_half(it, nh)))
                            for it in range(4 * s, 4 * s + 4)
                            for nh in range(2)
                        ]
                        backlog.extendleft(reversed(pr))

                ths.append((400, fin))
                return ths

            for s in (0, 1, 2, 3):
                cells = pass1(0, s)
                backlog.extend(make_pass2(0, s, cells))
            while not kq1_done[0]:
                emit_budget(1)
            for s in (3, 2, 1, 0):
                cells = pass1(1, s)
                backlog.extend(make_pass2(1, s, cells))
            while backlog:
                emit_budget(1)

    nc.compile()
    return nc


_NC = {}


def _get_nc(qk_bias=True):
    if qk_bias not in _NC:
        _NC[qk_bias] = _build(qk_bias=qk_bias)
    return _NC[qk_bias]


def _rope_tables():
    theta = (10000.0 ** (-np.arange(0, DH, 2, dtype=np.float32) / DH)).astype(
        np.float32
    )
    t = np.arange(T, dtype=np.float32)
    sinusoid = np.outer(t, theta).astype(np.float32)  # [T, DH/2]
    sin = np.concatenate([np.sin(sinusoid), np.sin(sinusoid)], axis=1)  # [T, DH]
    cos = np.concatenate([np.cos(sinusoid), np.cos(sinusoid)], axis=1)
    cosT = cos.T  # [DH, T]
    sinT = sin.T
    # sin_perm[e] = sin[(e+32) % 64]
    idx = (np.arange(DH) + 32) % DH
    sinTp = sinT[idx]
    cos2 = np.ascontiguousarray(np.concatenate([cosT, cosT], axis=0))  # [128, T]
    sinp2 = np.ascontiguousarray(np.concatenate([sinTp, sinTp], axis=0))
    return _bf(cos2), _bf(sinp2)


def _perm_matrix():
    p = np.zeros((128, 128), dtype=np.float32)
    for m in range(128):
        blk = m // 64
        k = blk * 64 + (m % 64 + 32) % 64
        p[k, m] = 1.0
    return p


def _mask_matrices():
    # maskT.T @ I adds -400 to S^T[k, q] where k > q (then exp(0.125*s)=0):
    # maskT[a, b] = -400 where b > a
    maskT = -400.0 * np.triu(np.ones((128, 128), dtype=np.float32), 1)
    return maskT, np.eye(128, dtype=np.float32)


def _bf(a):
    return np.ascontiguousarray(np.asarray(a, dtype=np.float32).astype(NPBF16))


def _prepare_in_maps(x, w_qkv, b_qkv, w_proj):
    x = np.asarray(x, dtype=np.float32)
    w_qkv = np.asarray(w_qkv, dtype=np.float32)
    b_qkv = np.asarray(b_qkv, dtype=np.float32)
    w_proj = np.asarray(w_proj, dtype=np.float32)

    cos2, sinp2 = _rope_tables()
    perm = _bf(_perm_matrix())
    maskT, id128 = _mask_matrices()
    maskT, id128 = _bf(maskT), _bf(id128)
    xTs = [_bf(x[b].T) for b in range(B)]

    in_maps = []
    for c in range(N_CORES):
        b, g = divmod(c, 4)
        h0 = g * GH  # first head of the group
        qcols = w_qkv[:, h0 * DH : (h0 + GH) * DH]
        kcols = w_qkv[:, C + h0 * DH : C + (h0 + GH) * DH]
        wqk = _bf(np.concatenate([qcols, kcols], axis=1))
        wv = np.zeros((C, VA), dtype=np.float32)
        bv = np.zeros((1, VA), dtype=np.float32)
        for j in range(GH):
            src = 2 * C + (h0 + j) * DH
            wv[:, j * 65 : j * 65 + DH] = w_qkv[:, src : src + DH]
            bv[0, j * 65 : j * 65 + DH] = b_qkv[src : src + DH]
            bv[0, j * 65 + DH] = 1.0
        bqk = np.concatenate(
            [b_qkv[h0 * DH : (h0 + GH) * DH], b_qkv[C + h0 * DH : C + (h0 + GH) * DH]]
        ).reshape(1, QK_COLS)
        wproj = np.stack(
            [w_proj[(h0 + 2 * p) * DH : (h0 + 2 * p + 2) * DH, :] for p in range(2)]
        )
        in_maps.append(
            {
                "xT": xTs[b],
                "wqk": wqk,
                "wv": _bf(wv),
                "bqk": _bf(bqk),
                "bv": _bf(bv),
                "cosT": cos2,
                "sinTp": sinp2,
                "perm": perm,
                "maskT": maskT,
                "id128": id128,
                "wproj": _bf(wproj),
            }
        )
    return in_maps


def run(x, w_qkv, b_qkv, w_proj, b_proj, trace=False, tmpdir=None):
    qk_bias = bool(np.any(np.asarray(b_qkv, dtype=np.float32)[: 2 * C]))
    nc = _get_nc(qk_bias)
    in_maps = _prepare_in_maps(x, w_qkv, b_qkv, w_proj)
    res = run_bass_kernel_spmd(
        nc, in_maps, list(range(N_CORES)), trace=trace, tmpdir=tmpdir
    )
    b_proj = np.asarray(b_proj, dtype=np.float32)
    out = np.empty((B, T, C), dtype=np.float32)
    for b in range(B):
        acc = res.results[4 * b]["out"].astype(np.float32)
        for g in range(1, 4):
            acc = acc + res.results[4 * b + g]["out"].astype(np.float32)
        out[b] = acc + b_proj
    return out, res


def kernel(x, w_qkv, b_qkv, w_proj, b_proj):
    out, _ = run(x, w_qkv, b_qkv, w_proj, b_proj, trace=False)
    return out


# revision 24
# speedup vs baseline: 1.1782x; 1.0355x over previous
"""Multi-head causal attention with RoPE on 8 Trainium2 NeuronCores.

Reference computation (B=2, T=2048, C=1024, H=16, Dh=64, fp32):
    qkv = x @ w_qkv + b_qkv ; split q,k,v ; RoPE(q), RoPE(k)
    attn = softmax_causal(q k^T / sqrt(Dh)) @ v ; out = attn @ w_proj + b_proj

Sharding: core c = b*4 + g handles batch b and head group g (heads 4g..4g+3).
Data-parallel over batch, tensor-parallel over heads (w_qkv column-split,
w_proj row-split).  Each core emits a partial [T, C] projection output; the
host sums the 4 per-batch partials and adds b_proj.

Per-core kernel, v2.  Heavy matmuls in bf16 (fp32 PSUM accumulation).
Design is driven by two trace findings on the v1 kernel: (a) the scalar
engine is the serial bottleneck of the attention phase (exp of all causal
scores at 1 elem/cycle/lane), and (b) the PE HAM clock gate throttled the
PE to 1.2 GHz for the whole attention phase (fp32r broadcast matmuls +
ACT table reloads for the Ln/Exp reciprocal created PE idle gaps).

  - The program is emitted as one interleaved stream: while the ACT-bound
    attention chunk pipeline of head pair 0 runs, the PE executes "filler"
    matmuls (V projection tiles, QKV+RoPE for pair 1) from the same queue;
    during pair 1's attention the fillers are the output-projection tiles
    of already-normalized spans.  The PE never idles long enough to drop
    to the throttled clock.
  - S^T tile = K_j Q^T per head; the two heads of a pair are emitted
    back-to-back with K=64 stationaries at row groups 0/64 (tile_position
    auto-derived from base partitions), so they run concurrently in the
    two halves of the PE array.
  - softmax: exp on ACT with the 1/sqrt(Dh) scale fused; causal via
    narrowing each k-tile's q-range plus one triangular -400 mask matmul
    on the diagonal 128x128 block.
  - V is augmented with a ones column so PV also emits the softmax
    denominator; 1/denom via vector.reciprocal_approx_fast (no ACT table
    switch), broadcast across partitions with a stride-0-source DMA, and
    applied by DVE as the PSUM->SBUF move of the attention tile.
  - projection: per head-pair stationary attn^T tiles vs w_proj rows,
    emitted per 512-column half so each PSUM tile is one bank.
"""

import numpy as np
import ml_dtypes

from collections import deque

import concourse.bacc as bacc
import concourse.bass as bass
import concourse.mybir as mybir
from concourse.tile import TileContext
from concourse.bass_utils import run_bass_kernel_spmd

F32 = mybir.dt.float32
BF16 = mybir.dt.bfloat16
NPBF16 = np.dtype(ml_dtypes.bfloat16)

B, T, C = 2, 2048, 1024
H, DH = 16, 64
GH = 4  # heads per core
N_CORES = 8
NCHUNK = C // 128  # 8 contraction chunks
NT = T // 128  # 16 token tiles
NSPAN = T // 512  # 4 query spans
QK_COLS = 2 * GH * DH  # 512 = q cols (256) + k cols (256)
VA = GH * (DH + 1)  # 260 = v cols augmented with ones column per head
EXP = mybir.ActivationFunctionType.Exp


def _build(qk_bias=True):
    nc = bacc.Bacc("TRN2", target_bir_lowering=False, debug=False, num_devices=N_CORES)

    xT = nc.dram_tensor("xT", [C, T], BF16, kind="ExternalInput")
    wqk = nc.dram_tensor("wqk", [C, QK_COLS], BF16, kind="ExternalInput")
    wv = nc.dram_tensor("wv", [C, VA], BF16, kind="ExternalInput")
    bqk_d = nc.dram_tensor("bqk", [1, QK_COLS], BF16, kind="ExternalInput")
    bv_d = nc.dram_tensor("bv", [1, VA], BF16, kind="ExternalInput")
    cos_d = nc.dram_tensor("cosT", [128, T], BF16, kind="ExternalInput")
    sinp_d = nc.dram_tensor("sinTp", [128, T], BF16, kind="ExternalInput")
    perm_d = nc.dram_tensor("perm", [128, 128], BF16, kind="ExternalInput")
    maskT_d = nc.dram_tensor("maskT", [128, 128], BF16, kind="ExternalInput")
    id_d = nc.dram_tensor("id128", [128, 128], BF16, kind="ExternalInput")
    wproj_d = nc.dram_tensor("wproj", [2, 128, C], BF16, kind="ExternalInput")
    out_d = nc.dram_tensor("out", [T, C], BF16, kind="ExternalOutput")

    with TileContext(nc) as tc:
        with (
            tc.tile_pool(name="pers", bufs=1) as pers,
            tc.tile_pool(name="ps_s", bufs=2, space="PSUM") as ps_s,
            tc.tile_pool(name="ps_pv", bufs=2, space="PSUM") as ps_pv,
            tc.tile_pool(name="ps_fill", bufs=2, space="PSUM") as ps_fill,
            tc.tile_pool(name="sbw", bufs=1) as sbw,
        ):
            ones = pers.tile([1, 512], BF16, tag="ones")
            nc.vector.memset(ones, 1.0)
            warm = pers.tile([1, 8], F32, tag="warm")
            # Prepay the exp ACT-table load during the DMA ramp.
            nc.scalar.activation(out=warm, in_=ones[0:1, 0:8], func=EXP, scale=0.125)

            # ---------------- input DMA (multi-queue) --------------------
            xt = []
            for kc in range(NCHUNK):
                t = pers.tile([128, T], BF16, tag="xt", bufs=NCHUNK, name=f"xt{kc}")
                eng = nc.sync if kc % 2 == 0 else nc.scalar
                eng.dma_start(out=t, in_=xT[128 * kc : 128 * (kc + 1), :])
                xt.append(t)
            wqk_t = []
            for kc in range(NCHUNK):
                t = pers.tile(
                    [128, QK_COLS], BF16, tag="wqk", bufs=NCHUNK, name=f"wqk{kc}"
                )
                nc.gpsimd.dma_start(out=t, in_=wqk[128 * kc : 128 * (kc + 1), :])
                wqk_t.append(t)
            bqk_sb = pers.tile([1, QK_COLS], BF16, tag="bqk")
            nc.sync.dma_start(out=bqk_sb, in_=bqk_d[:, :])
            bv_sb = pers.tile([1, VA], BF16, tag="bv")
            nc.sync.dma_start(out=bv_sb, in_=bv_d[:, :])
            perm_sb = pers.tile([128, 128], BF16, tag="perm")
            nc.sync.dma_start(out=perm_sb, in_=perm_d[:, :])
            mask_sb = pers.tile([128, 128], BF16, tag="maskT")
            nc.sync.dma_start(out=mask_sb, in_=maskT_d[:, :])
            id_sb = pers.tile([128, 128], BF16, tag="id128")
            nc.sync.dma_start(out=id_sb, in_=id_d[:, :])
            wv_t = []
            for kc in range(NCHUNK):
                t = pers.tile([128, VA], BF16, tag="wv", bufs=NCHUNK, name=f"wv{kc}")
                nc.gpsimd.dma_start(out=t, in_=wv[128 * kc : 128 * (kc + 1), :])
                wv_t.append(t)
            cos_sb = pers.tile([128, T], BF16, tag="cos")
            nc.gpsimd.dma_start(out=cos_sb, in_=cos_d[:, :])
            sinp_sb = pers.tile([128, T], BF16, tag="sinp")
            nc.gpsimd.dma_start(out=sinp_sb, in_=sinp_d[:, :])
            wproj_sb = []
            for p in range(2):
                t = pers.tile([128, C], BF16, tag="wproj", bufs=2, name=f"wproj{p}")
                nc.scalar.dma_start(out=t, in_=wproj_d[p, :, :])
                wproj_sb.append(t)

            # Persistent intermediate tiles
            qkt = []  # 4 tiles [128, T]: Q heads(0,1), Q(2,3), K(0,1), K(2,3)
            for i in range(4):
                t = pers.tile([128, T], BF16, tag="qkt", bufs=4, name=f"qkt{i}")
                qkt.append(t)
            vaug = []  # 16 tiles [128, VA], k-tile-major natural layout V
            for j in range(NT):
                t = pers.tile([128, VA], BF16, tag="vaug", bufs=NT, name=f"vaug{j}")
                vaug.append(t)
            attn = []  # 2 tiles [128, T]: normalized attn^T for head pairs
            for p in range(2):
                t = pers.tile([128, T], BF16, tag="attn", bufs=2, name=f"attn{p}")
                attn.append(t)

            # ---------------- emission helpers ---------------------------
            def rope(ct, sp, pq, permpool, permtag):
                # qkt[ct][:, ss] = pq*cos + perm @ (pq*sin_perm)
                ss = slice(512 * sp, 512 * (sp + 1))
                t2 = sbw.tile([128, 512], BF16, tag="t2", bufs=3, name="t2")
                nc.vector.tensor_mul(t2, pq, sinp_sb[:, ss])
                pp = permpool.tile([128, 512], F32, tag=permtag, name="psperm")
                nc.tensor.matmul(pp, perm_sb, t2, start=True, stop=True)
                nc.vector.tensor_mul(qkt[ct][:, ss], pq, cos_sb[:, ss])
                nc.vector.tensor_add(qkt[ct][:, ss], qkt[ct][:, ss], pp)


            def qk_bias_mm(tile, cs):
                if qk_bias:
                    nc.tensor.matmul(
                        tile, bqk_sb[0:1, cs], ones, start=False, stop=True
                    )

            def qk_part(ct, sps, pool, tag, chunked):
                # Q-or-K column tile for a pair of spans, kc-inner (chasing
                # the x DMA), fused bias + RoPE at the end.
                cs = slice(128 * ct, 128 * (ct + 1))
                if tag == "s":
                    big = pool.tile([128, 1024], F32, tag="s", name="psqkA")
                    tiles = {
                        sp: big[:, 512 * i : 512 * i + 512]
                        for i, sp in enumerate(sps)
                    }
                else:
                    tiles = {
                        sp: pool.tile([128, 512], F32, tag=tag, name="psqk")
                        for sp in sps
                    }
                for kc in range(NCHUNK):
                    for sp in sps:
                        nc.tensor.matmul(
                            tiles[sp],
                            wqk_t[kc][:, cs],
                            xt[kc][:, 512 * sp : 512 * (sp + 1)],
                            start=(kc == 0),
                            stop=(not qk_bias) and kc == NCHUNK - 1,
                        )
                    if chunked:
                        yield
                for sp in sps:
                    qk_bias_mm(tiles[sp], cs)
                    rope(ct, sp, tiles[sp], ps_fill, "ps_fill")
                    if chunked:
                        yield

            def v_tile(it, chunked):
                pv = ps_fill.tile([128, VA], F32, tag="ps_fill", name="psv")
                ts = slice(128 * it, 128 * (it + 1))
                for kc in range(NCHUNK):
                    nc.tensor.matmul(
                        pv, xt[kc][:, ts], wv_t[kc], start=(kc == 0), stop=False
                    )
                    if chunked and kc % 2 == 1 and kc < 7:
                        yield
                nc.tensor.matmul(pv, ones[0:1, 0:128], bv_sb, start=False, stop=True)
                nc.vector.tensor_copy(vaug[it], pv)
                if chunked:
                    yield

            def normalize(pair, idx, s, pv):
                # attn = pv[0:64] * (1/colsum).  The denominator (ones-column
                # PV output, PSUM row 64) is copied to partition 0 — the
                # custom-DVE reciprocal only works at base partition 0 — then
                # broadcast across partitions on GPSIMD.  No ACT tables, no
                # fp32r matmuls (both throttled the v1 kernel).
                po = idx * 64
                ss = slice(512 * s, 512 * (s + 1))
                d0 = sbw.tile([1, 512], F32, tag="d0", bufs=2, name="d0")
                nc.vector.tensor_copy(d0, pv[64:65, :])
                r = sbw.tile([1, 512], F32, tag="r", bufs=2, name="r")
                nc.vector.reciprocal_approx_fast(out=r, in_=d0)
                rbs = sbw.tile([64, 512], F32, tag="rbs", bufs=3, name="rbs")
                nc.gpsimd.partition_broadcast(rbs, r)
                nc.vector.tensor_mul(attn[pair][po : po + 64, ss], pv[0:64, :], rbs)

            def proj_half(it, nh):
                # out[ts, ns] = sum_p attn[p][:, ts]^T @ wproj[p][:, ns]
                ts = slice(128 * it, 128 * (it + 1))
                ns = slice(512 * nh, 512 * (nh + 1))
                ppj = ps_fill.tile([128, 512], F32, tag="ps_fill", name="psproj")
                for p in range(2):
                    nc.tensor.matmul(
                        ppj,
                        attn[p][:, ts],
                        wproj_sb[p][:, ns],
                        start=(p == 0),
                        stop=(p == 1),
                    )
                ob = sbw.tile([128, 512], BF16, tag="ob", bufs=4, name="ob")
                if it < 8:
                    # spans 0-1 of pair 1 are projected after the last exp;
                    # the then-idle ACT takes their PSUM evacuation
                    nc.scalar.copy(ob, ppj)
                else:
                    nc.vector.tensor_copy(ob, ppj)
                eng = nc.sync if (it + nh) % 2 == 0 else nc.scalar
                eng.dma_start(out=out_d[ts, ns], in_=ob)

            # ---------------- phase A: dense PE ramp ---------------------
            # only spans 0-1 of K and Q for pair 0 (all pass1(0,0)/(0,1)
            # needs); spans 2-3 are backlog so attention starts right after
            # the x DMA completes.
            for _ in qk_part(2, (0, 1), ps_s, "s", False):
                pass
            for _ in qk_part(0, (0, 1), ps_s, "s", False):
                pass

            # ------------- two-phase attention with a PE backlog ---------
            # pass1(pair, s): S + fused exp per k-tile, et tiles -> SBUF.
            # pass2(pair, s): PV + normalize, emitted later as backlog
            # thunks between pass1 steps so the PE always has dense,
            # ACT-independent work (the PE only reaches 2.4 GHz after ~3us
            # of continuous execution; any stall resets it to 1.2 GHz).
            backlog = deque()

            def emit_budget(budget):
                while budget > 0 and backlog:
                    cost, fn = backlog.popleft()
                    fn()
                    budget -= cost
                return budget

            def gen_thunks(gen, n, cost):
                return [(cost, (lambda g=gen: next(g, None))) for _ in range(n)]

            kq1_done = [False]
            sp23_done = [False]

            def mark_kq1():
                kq1_done[0] = True

            def mark_sp23():
                sp23_done[0] = True

            backlog.extend(
                gen_thunks(qk_part(2, (2, 3), ps_fill, "ps_fill", True), 10, 1024)
            )
            backlog.extend(
                gen_thunks(qk_part(0, (2, 3), ps_fill, "ps_fill", True), 10, 1024)
            )
            backlog.append((0, mark_sp23))
            backlog.extend(
                th for it in range(NT) for th in gen_thunks(v_tile(it, True), 4, 550)
            )
            for ct in (3, 1):
                for sps in ((0, 1), (2, 3)):
                    backlog.extend(
                        gen_thunks(qk_part(ct, sps, ps_fill, "ps_fill", True), 10, 1024)
                    )
            backlog.append((0, mark_kq1))

            def pass1(pair, s):
                qt, kt = qkt[pair], qkt[2 + pair]
                cells = []
                for j in range(4 * s + 4):
                    st = ps_s.tile([128, 1024], F32, tag="s", name="st")
                    q0 = max(512 * s, 128 * j)
                    w = 512 * (s + 1) - q0
                    diag = s == j // 4
                    for idx in (0, 1):
                        po = idx * 64
                        nc.tensor.matmul(
                            st[:, 512 * idx : 512 * idx + w],
                            kt[po : po + 64, 128 * j : 128 * (j + 1)],
                            qt[po : po + 64, q0 : q0 + w],
                            start=True,
                            stop=not diag,
                        )
                    if diag:
                        for idx in (0, 1):
                            nc.tensor.matmul(
                                st[:, 512 * idx : 512 * idx + 128],
                                mask_sb,
                                id_sb,
                                start=False,
                                stop=True,
                            )
                    et = sbw.tile([128, 1024], BF16, tag="et", bufs=36, name="et")
                    if w == 512:
                        nc.scalar.activation(
                            out=et[:, :], in_=st[:, :], func=EXP, scale=0.125
                        )
                    else:
                        iv = st[:, :].rearrange("p (h c) -> p h c", h=2)[:, :, 0:w]
                        ov = et[:, :].rearrange("p (h c) -> p h c", h=2)[:, :, 0:w]
                        nc.scalar.activation(out=ov, in_=iv, func=EXP, scale=0.125)
                    cells.append((j, q0, w, et))
                    emit_budget((500 + 2 * w) if pair == 0 else (900 + 3 * w))
                return cells

            def make_pass2(pair, s, cells):
                heads = (2 * pair, 2 * pair + 1)
                hold = {}
                ths = []
                for i, (j, q0, w, et) in enumerate(cells):
                    def th(i=i, j=j, q0=q0, w=w, et=et):
                        if i == 0:
                            hold["pv"] = [
                                ps_pv.tile([65, 512], F32, tag="pv", name=f"pspv{k}")
                                for k in (0, 1)
                            ]
                        for idx in (0, 1):
                            h = heads[idx]
                            nc.tensor.matmul(
                                hold["pv"][idx][:, q0 - 512 * s :],
                                vaug[j][:, 65 * h : 65 * (h + 1)],
                                et[:, 512 * idx : 512 * idx + w],
                                start=(j == 0),
                                stop=(j == 4 * s + 3),
                            )
                    ths.append((2 * w, th))

                def fin():
                    for idx in (0, 1):
                        normalize(pair, idx, s, hold["pv"][idx])
                    if pair == 1:
                        # proj right behind the normalize it depends on, so
                        # the PE has work while the normalize chain runs
                        pr = [
                            (1300, (lambda it=it, nh=nh: proj_half(it, nh)))
                            for it in range(4 * s, 4 * s + 4)
                            for nh in range(2)
                        ]
                        backlog.extendleft(reversed(pr))

                ths.append((400, fin))
                return ths

            for s in (0, 1, 2, 3):
                if s == 2:
                    while not sp23_done[0]:
                        emit_budget(1)
                cells = pass1(0, s)
                backlog.extend(make_pass2(0, s, cells))
            while not kq1_done[0]:
                emit_budget(1)
            for s in (3, 2, 1, 0):
                cells = pass1(1, s)
                backlog.extend(make_pass2(1, s, cells))
            while backlog:
                emit_budget(1)

    nc.compile()
    return nc


_NC = {}


def _get_nc(qk_bias=True):
    if qk_bias not in _NC:
        _NC[qk_bias] = _build(qk_bias=qk_bias)
    return _NC[qk_bias]


def _rope_tables():
    theta = (10000.0 ** (-np.arange(0, DH, 2, dtype=np.float32) / DH)).astype(
        np.float32
    )
    t = np.arange(T, dtype=np.float32)
    sinusoid = np.outer(t, theta).astype(np.float32)  # [T, DH/2]
    sin = np.concatenate([np.sin(sinusoid), np.sin(sinusoid)], axis=1)  # [T, DH]
    cos = np.concatenate([np.cos(sinusoid), np.cos(sinusoid)], axis=1)
    cosT = cos.T  # [DH, T]
    sinT = sin.T
    # sin_perm[e] = sin[(e+32) % 64]
    idx = (np.arange(DH) + 32) % DH
    sinTp = sinT[idx]
    cos2 = np.ascontiguousarray(np.concatenate([cosT, cosT], axis=0))  # [128, T]
    sinp2 = np.ascontiguousarray(np.concatenate([sinTp, sinTp], axis=0))
    return _bf(cos2), _bf(sinp2)


def _perm_matrix():
    p = np.zeros((128, 128), dtype=np.float32)
    for m in range(128):
        blk = m // 64
        k = blk * 64 + (m % 64 + 32) % 64
        p[k, m] = 1.0
    return p


def _mask_matrices():
    # maskT.T @ I adds -400 to S^T[k, q] where k > q (then exp(0.125*s)=0):
    # maskT[a, b] = -400 where b > a
    maskT = -400.0 * np.triu(np.ones((128, 128), dtype=np.float32), 1)
    return maskT, np.eye(128, dtype=np.float32)


def _bf(a):
    return np.ascontiguousarray(np.asarray(a, dtype=np.float32).astype(NPBF16))


def _prepare_in_maps(x, w_qkv, b_qkv, w_proj):
    x = np.asarray(x, dtype=np.float32)
    w_qkv = np.asarray(w_qkv, dtype=np.float32)
    b_qkv = np.asarray(b_qkv, dtype=np.float32)
    w_proj = np.asarray(w_proj, dtype=np.float32)

    cos2, sinp2 = _rope_tables()
    perm = _bf(_perm_matrix())
    maskT, id128 = _mask_matrices()
    maskT, id128 = _bf(maskT), _bf(id128)
    xTs = [_bf(x[b].T) for b in range(B)]

    in_maps = []
    for c in range(N_CORES):
        b, g = divmod(c, 4)
        h0 = g * GH  # first head of the group
        qcols = w_qkv[:, h0 * DH : (h0 + GH) * DH]
        kcols = w_qkv[:, C + h0 * DH : C + (h0 + GH) * DH]
        wqk = _bf(np.concatenate([qcols, kcols], axis=1))
        wv = np.zeros((C, VA), dtype=np.float32)
        bv = np.zeros((1, VA), dtype=np.float32)
        for j in range(GH):
            src = 2 * C + (h0 + j) * DH
            wv[:, j * 65 : j * 65 + DH] = w_qkv[:, src : src + DH]
            bv[0, j * 65 : j * 65 + DH] = b_qkv[src : src + DH]
            bv[0, j * 65 + DH] = 1.0
        bqk = np.concatenate(
            [b_qkv[h0 * DH : (h0 + GH) * DH], b_qkv[C + h0 * DH : C + (h0 + GH) * DH]]
        ).reshape(1, QK_COLS)
        wproj = np.stack(
            [w_proj[(h0 + 2 * p) * DH : (h0 + 2 * p + 2) * DH, :] for p in range(2)]
        )
        in_maps.append(
            {
                "xT": xTs[b],
                "wqk": wqk,
                "wv": _bf(wv),
                "bqk": _bf(bqk),
                "bv": _bf(bv),
                "cosT": cos2,
                "sinTp": sinp2,
                "perm": perm,
                "maskT": maskT,
                "id128": id128,
                "wproj": _bf(wproj),
            }
        )
    return in_maps


def run(x, w_qkv, b_qkv, w_proj, b_proj, trace=False, tmpdir=None):
    qk_bias = bool(np.any(np.asarray(b_qkv, dtype=np.float32)[: 2 * C]))
    nc = _get_nc(qk_bias)
    in_maps = _prepare_in_maps(x, w_qkv, b_qkv, w_proj)
    res = run_bass_kernel_spmd(
        nc, in_maps, list(range(N_CORES)), trace=trace, tmpdir=tmpdir
    )
    b_proj = np.asarray(b_proj, dtype=np.float32)
    out = np.empty((B, T, C), dtype=np.float32)
    for b in range(B):
        acc = res.results[4 * b]["out"].astype(np.float32)
        for g in range(1, 4):
            acc = acc + res.results[4 * b + g]["out"].astype(np.float32)
        out[b] = acc + b_proj
    return out, res


def kernel(x, w_qkv, b_qkv, w_proj, b_proj):
    out, _ = run(x, w_qkv, b_qkv, w_proj, b_proj, trace=False)
    return out


# revision 25
# speedup vs baseline: 1.1831x; 1.0041x over previous
"""Multi-head causal attention with RoPE on 8 Trainium2 NeuronCores.

Reference computation (B=2, T=2048, C=1024, H=16, Dh=64, fp32):
    qkv = x @ w_qkv + b_qkv ; split q,k,v ; RoPE(q), RoPE(k)
    attn = softmax_causal(q k^T / sqrt(Dh)) @ v ; out = attn @ w_proj + b_proj

Sharding: core c = b*4 + g handles batch b and head group g (heads 4g..4g+3).
Data-parallel over batch, tensor-parallel over heads (w_qkv column-split,
w_proj row-split).  Each core emits a partial [T, C] projection output; the
host sums the 4 per-batch partials and adds b_proj.

Per-core kernel, v2.  Heavy matmuls in bf16 (fp32 PSUM accumulation).
Design is driven by two trace findings on the v1 kernel: (a) the scalar
engine is the serial bottleneck of the attention phase (exp of all causal
scores at 1 elem/cycle/lane), and (b) the PE HAM clock gate throttled the
PE to 1.2 GHz for the whole attention phase (fp32r broadcast matmuls +
ACT table reloads for the Ln/Exp reciprocal created PE idle gaps).

  - The program is emitted as one interleaved stream: while the ACT-bound
    attention chunk pipeline of head pair 0 runs, the PE executes "filler"
    matmuls (V projection tiles, QKV+RoPE for pair 1) from the same queue;
    during pair 1's attention the fillers are the output-projection tiles
    of already-normalized spans.  The PE never idles long enough to drop
    to the throttled clock.
  - S^T tile = K_j Q^T per head; the two heads of a pair are emitted
    back-to-back with K=64 stationaries at row groups 0/64 (tile_position
    auto-derived from base partitions), so they run concurrently in the
    two halves of the PE array.
  - softmax: exp on ACT with the 1/sqrt(Dh) scale fused; causal via
    narrowing each k-tile's q-range plus one triangular -400 mask matmul
    on the diagonal 128x128 block.
  - V is augmented with a ones column so PV also emits the softmax
    denominator; 1/denom via vector.reciprocal_approx_fast (no ACT table
    switch), broadcast across partitions with a stride-0-source DMA, and
    applied by DVE as the PSUM->SBUF move of the attention tile.
  - projection: per head-pair stationary attn^T tiles vs w_proj rows,
    emitted per 512-column half so each PSUM tile is one bank.
"""

import numpy as np
import ml_dtypes

from collections import deque

import concourse.bacc as bacc
import concourse.bass as bass
import concourse.mybir as mybir
from concourse.tile import TileContext
from concourse.bass_utils import run_bass_kernel_spmd

F32 = mybir.dt.float32
BF16 = mybir.dt.bfloat16
NPBF16 = np.dtype(ml_dtypes.bfloat16)

B, T, C = 2, 2048, 1024
H, DH = 16, 64
GH = 4  # heads per core
N_CORES = 8
NCHUNK = C // 128  # 8 contraction chunks
NT = T // 128  # 16 token tiles
NSPAN = T // 512  # 4 query spans
QK_COLS = 2 * GH * DH  # 512 = q cols (256) + k cols (256)
VA = GH * (DH + 1)  # 260 = v cols augmented with ones column per head
EXP = mybir.ActivationFunctionType.Exp


def _build(qk_bias=True):
    nc = bacc.Bacc("TRN2", target_bir_lowering=False, debug=False, num_devices=N_CORES)

    xT = nc.dram_tensor("xT", [C, T], BF16, kind="ExternalInput")
    wqk = nc.dram_tensor("wqk", [C, QK_COLS], BF16, kind="ExternalInput")
    wv = nc.dram_tensor("wv", [C, VA], BF16, kind="ExternalInput")
    bqk_d = nc.dram_tensor("bqk", [1, QK_COLS], BF16, kind="ExternalInput")
    bv_d = nc.dram_tensor("bv", [1, VA], BF16, kind="ExternalInput")
    cos_d = nc.dram_tensor("cosT", [128, T], BF16, kind="ExternalInput")
    sinp_d = nc.dram_tensor("sinTp", [128, T], BF16, kind="ExternalInput")
    perm_d = nc.dram_tensor("perm", [128, 128], BF16, kind="ExternalInput")
    maskT_d = nc.dram_tensor("maskT", [128, 128], BF16, kind="ExternalInput")
    id_d = nc.dram_tensor("id128", [128, 128], BF16, kind="ExternalInput")
    wproj_d = nc.dram_tensor("wproj", [2, 128, C], BF16, kind="ExternalInput")
    out_d = nc.dram_tensor("out", [T, C], BF16, kind="ExternalOutput")

    with TileContext(nc) as tc:
        with (
            tc.tile_pool(name="pers", bufs=1) as pers,
            tc.tile_pool(name="ps_s", bufs=2, space="PSUM") as ps_s,
            tc.tile_pool(name="ps_pv", bufs=2, space="PSUM") as ps_pv,
            tc.tile_pool(name="ps_fill", bufs=2, space="PSUM") as ps_fill,
            tc.tile_pool(name="sbw", bufs=1) as sbw,
        ):
            ones = pers.tile([1, 512], BF16, tag="ones")
            nc.vector.memset(ones, 1.0)
            warm = pers.tile([1, 8], F32, tag="warm")
            # Prepay the exp ACT-table load during the DMA ramp.
            nc.scalar.activation(out=warm, in_=ones[0:1, 0:8], func=EXP, scale=0.125)

            # ---------------- input DMA (multi-queue) --------------------
            xt = []
            for kc in range(NCHUNK):
                t = pers.tile([128, T], BF16, tag="xt", bufs=NCHUNK, name=f"xt{kc}")
                eng = nc.sync if kc % 2 == 0 else nc.scalar
                eng.dma_start(out=t, in_=xT[128 * kc : 128 * (kc + 1), :])
                xt.append(t)
            wqk_t = []
            for kc in range(NCHUNK):
                t = pers.tile(
                    [128, QK_COLS], BF16, tag="wqk", bufs=NCHUNK, name=f"wqk{kc}"
                )
                nc.gpsimd.dma_start(out=t, in_=wqk[128 * kc : 128 * (kc + 1), :])
                wqk_t.append(t)
            bqk_sb = pers.tile([1, QK_COLS], BF16, tag="bqk")
            nc.sync.dma_start(out=bqk_sb, in_=bqk_d[:, :])
            bv_sb = pers.tile([1, VA], BF16, tag="bv")
            nc.sync.dma_start(out=bv_sb, in_=bv_d[:, :])
            perm_sb = pers.tile([128, 128], BF16, tag="perm")
            nc.sync.dma_start(out=perm_sb, in_=perm_d[:, :])
            mask_sb = pers.tile([128, 128], BF16, tag="maskT")
            nc.sync.dma_start(out=mask_sb, in_=maskT_d[:, :])
            id_sb = pers.tile([128, 128], BF16, tag="id128")
            nc.sync.dma_start(out=id_sb, in_=id_d[:, :])
            wv_t = []
            for kc in range(NCHUNK):
                t = pers.tile([128, VA], BF16, tag="wv", bufs=NCHUNK, name=f"wv{kc}")
                nc.gpsimd.dma_start(out=t, in_=wv[128 * kc : 128 * (kc + 1), :])
                wv_t.append(t)
            cos_sb = pers.tile([128, T], BF16, tag="cos")
            nc.gpsimd.dma_start(out=cos_sb, in_=cos_d[:, :])
            sinp_sb = pers.tile([128, T], BF16, tag="sinp")
            nc.gpsimd.dma_start(out=sinp_sb, in_=sinp_d[:, :])
            wproj_sb = []
            for p in range(2):
                t = pers.tile([128, C], BF16, tag="wproj", bufs=2, name=f"wproj{p}")
                nc.scalar.dma_start(out=t, in_=wproj_d[p, :, :])
                wproj_sb.append(t)

            # Persistent intermediate tiles
            qkt = []  # 4 tiles [128, T]: Q heads(0,1), Q(2,3), K(0,1), K(2,3)
            for i in range(4):
                t = pers.tile([128, T], BF16, tag="qkt", bufs=4, name=f"qkt{i}")
                qkt.append(t)
            vaug = []  # 16 tiles [128, VA], k-tile-major natural layout V
            for j in range(NT):
                t = pers.tile([128, VA], BF16, tag="vaug", bufs=NT, name=f"vaug{j}")
                vaug.append(t)
            attn = []  # 2 tiles [128, T]: normalized attn^T for head pairs
            for p in range(2):
                t = pers.tile([128, T], BF16, tag="attn", bufs=2, name=f"attn{p}")
                attn.append(t)

            # ---------------- emission helpers ---------------------------
            def rope(ct, sp, pq, permpool, permtag):
                # qkt[ct][:, ss] = pq*cos + perm @ (pq*sin_perm)
                ss = slice(512 * sp, 512 * (sp + 1))
                t2 = sbw.tile([128, 512], BF16, tag="t2", bufs=3, name="t2")
                nc.vector.tensor_mul(t2, pq, sinp_sb[:, ss])
                pp = permpool.tile([128, 512], F32, tag=permtag, name="psperm")
                nc.tensor.matmul(pp, perm_sb, t2, start=True, stop=True)
                nc.vector.tensor_mul(qkt[ct][:, ss], pq, cos_sb[:, ss])
                nc.vector.tensor_add(qkt[ct][:, ss], qkt[ct][:, ss], pp)


            def qk_bias_mm(tile, cs):
                if qk_bias:
                    nc.tensor.matmul(
                        tile, bqk_sb[0:1, cs], ones, start=False, stop=True
                    )

            def qk_part(ct, sps, pool, tag, chunked):
                # Q-or-K column tile for a pair of spans, kc-inner (chasing
                # the x DMA), fused bias + RoPE at the end.
                cs = slice(128 * ct, 128 * (ct + 1))
                if tag == "s":
                    big = pool.tile([128, 1024], F32, tag="s", name="psqkA")
                    tiles = {
                        sp: big[:, 512 * i : 512 * i + 512]
                        for i, sp in enumerate(sps)
                    }
                else:
                    tiles = {
                        sp: pool.tile([128, 512], F32, tag=tag, name="psqk")
                        for sp in sps
                    }
                for kc in range(NCHUNK):
                    for sp in sps:
                        nc.tensor.matmul(
                            tiles[sp],
                            wqk_t[kc][:, cs],
                            xt[kc][:, 512 * sp : 512 * (sp + 1)],
                            start=(kc == 0),
                            stop=(not qk_bias) and kc == NCHUNK - 1,
                        )
                    if chunked:
                        yield
                for sp in sps:
                    qk_bias_mm(tiles[sp], cs)
                    rope(ct, sp, tiles[sp], ps_fill, "ps_fill")
                    if chunked:
                        yield

            def v_tile(it, chunked):
                pv = ps_fill.tile([128, VA], F32, tag="ps_fill", name="psv")
                ts = slice(128 * it, 128 * (it + 1))
                for kc in range(NCHUNK):
                    nc.tensor.matmul(
                        pv, xt[kc][:, ts], wv_t[kc], start=(kc == 0), stop=False
                    )
                    if chunked and kc % 2 == 1 and kc < 7:
                        yield
                nc.tensor.matmul(pv, ones[0:1, 0:128], bv_sb, start=False, stop=True)
                nc.vector.tensor_copy(vaug[it], pv)
                if chunked:
                    yield

            def normalize(pair, idx, s, pv):
                # attn = pv[0:64] * (1/colsum).  The denominator (ones-column
                # PV output, PSUM row 64) is copied to partition 0 — the
                # custom-DVE reciprocal only works at base partition 0 — then
                # broadcast across partitions on GPSIMD.  No ACT tables, no
                # fp32r matmuls (both throttled the v1 kernel).
                po = idx * 64
                ss = slice(512 * s, 512 * (s + 1))
                d0 = sbw.tile([1, 512], F32, tag="d0", bufs=2, name="d0")
                nc.vector.tensor_copy(d0, pv[64:65, :])
                r = sbw.tile([1, 512], F32, tag="r", bufs=2, name="r")
                nc.vector.reciprocal_approx_fast(out=r, in_=d0)
                rbs = sbw.tile([64, 512], F32, tag="rbs", bufs=3, name="rbs")
                nc.gpsimd.partition_broadcast(rbs, r)
                nc.vector.tensor_mul(attn[pair][po : po + 64, ss], pv[0:64, :], rbs)

            def proj_half(it, nh):
                # out[ts, ns] = sum_p attn[p][:, ts]^T @ wproj[p][:, ns]
                ts = slice(128 * it, 128 * (it + 1))
                ns = slice(512 * nh, 512 * (nh + 1))
                ppj = ps_fill.tile([128, 512], F32, tag="ps_fill", name="psproj")
                for p in range(2):
                    nc.tensor.matmul(
                        ppj,
                        attn[p][:, ts],
                        wproj_sb[p][:, ns],
                        start=(p == 0),
                        stop=(p == 1),
                    )
                ob = sbw.tile([128, 512], BF16, tag="ob", bufs=4, name="ob")
                if it < 8:
                    # spans 0-1 of pair 1 are projected after the last exp;
                    # the then-idle ACT takes their PSUM evacuation
                    nc.scalar.copy(ob, ppj)
                else:
                    nc.vector.tensor_copy(ob, ppj)
                eng = nc.sync if (it + nh) % 2 == 0 else nc.scalar
                eng.dma_start(out=out_d[ts, ns], in_=ob)

            # ---------------- phase A: dense PE ramp ---------------------
            # only spans 0-1 of K and Q for pair 0 (all pass1(0,0)/(0,1)
            # needs); spans 2-3 are backlog so attention starts right after
            # the x DMA completes.
            for _ in qk_part(2, (0, 1), ps_s, "s", False):
                pass
            for _ in qk_part(0, (0, 1), ps_s, "s", False):
                pass

            # ------------- two-phase attention with a PE backlog ---------
            # pass1(pair, s): S + fused exp per k-tile, et tiles -> SBUF.
            # pass2(pair, s): PV + normalize, emitted later as backlog
            # thunks between pass1 steps so the PE always has dense,
            # ACT-independent work (the PE only reaches 2.4 GHz after ~3us
            # of continuous execution; any stall resets it to 1.2 GHz).
            backlog = deque()

            def emit_budget(budget):
                while budget > 0 and backlog:
                    cost, fn = backlog.popleft()
                    fn()
                    budget -= cost
                return budget

            def gen_thunks(gen, n, cost):
                return [(cost, (lambda g=gen: next(g, None))) for _ in range(n)]

            kq1_done = [False]
            sp23_done = [False]

            def mark_kq1():
                kq1_done[0] = True

            def mark_sp23():
                sp23_done[0] = True

            backlog.extend(
                gen_thunks(qk_part(2, (2, 3), ps_fill, "ps_fill", True), 10, 1024)
            )
            backlog.extend(
                gen_thunks(qk_part(0, (2, 3), ps_fill, "ps_fill", True), 10, 1024)
            )
            backlog.append((0, mark_sp23))
            backlog.extend(
                th for it in range(NT) for th in gen_thunks(v_tile(it, True), 4, 550)
            )
            for ct in (3, 1):
                for sps in ((0, 1), (2, 3)):
                    backlog.extend(
                        gen_thunks(qk_part(ct, sps, ps_fill, "ps_fill", True), 10, 1024)
                    )
            backlog.append((0, mark_kq1))

            def pass1(pair, s):
                qt, kt = qkt[pair], qkt[2 + pair]
                cells = []
                for j in range(4 * s + 4):
                    st = ps_s.tile([128, 1024], F32, tag="s", name="st")
                    q0 = max(512 * s, 128 * j)
                    w = 512 * (s + 1) - q0
                    diag = s == j // 4
                    for idx in (0, 1):
                        po = idx * 64
                        nc.tensor.matmul(
                            st[:, 512 * idx : 512 * idx + w],
                            kt[po : po + 64, 128 * j : 128 * (j + 1)],
                            qt[po : po + 64, q0 : q0 + w],
                            start=True,
                            stop=not diag,
                        )
                    if diag:
                        for idx in (0, 1):
                            nc.tensor.matmul(
                                st[:, 512 * idx : 512 * idx + 128],
                                mask_sb,
                                id_sb,
                                start=False,
                                stop=True,
                            )
                    et = sbw.tile([128, 1024], BF16, tag="et", bufs=36, name="et")
                    if w == 512:
                        nc.scalar.activation(
                            out=et[:, :], in_=st[:, :], func=EXP, scale=0.125
                        )
                    else:
                        iv = st[:, :].rearrange("p (h c) -> p h c", h=2)[:, :, 0:w]
                        ov = et[:, :].rearrange("p (h c) -> p h c", h=2)[:, :, 0:w]
                        nc.scalar.activation(out=ov, in_=iv, func=EXP, scale=0.125)
                    cells.append((j, q0, w, et))
                    emit_budget(900 + 3 * w)
                return cells

            def make_pass2(pair, s, cells):
                heads = (2 * pair, 2 * pair + 1)
                hold = {}
                ths = []
                for i, (j, q0, w, et) in enumerate(cells):
                    def th(i=i, j=j, q0=q0, w=w, et=et):
                        if i == 0:
                            hold["pv"] = [
                                ps_pv.tile([65, 512], F32, tag="pv", name=f"pspv{k}")
                                for k in (0, 1)
                            ]
                        for idx in (0, 1):
                            h = heads[idx]
                            nc.tensor.matmul(
                                hold["pv"][idx][:, q0 - 512 * s :],
                                vaug[j][:, 65 * h : 65 * (h + 1)],
                                et[:, 512 * idx : 512 * idx + w],
                                start=(j == 0),
                                stop=(j == 4 * s + 3),
                            )
                    ths.append((2 * w, th))

                def fin():
                    for idx in (0, 1):
                        normalize(pair, idx, s, hold["pv"][idx])
                    if pair == 1:
                        # proj right behind the normalize it depends on, so
                        # the PE has work while the normalize chain runs
                        pr = [
                            (1300, (lambda it=it, nh=nh: proj_half(it, nh)))
                            for it in range(4 * s, 4 * s + 4)
                            for nh in range(2)
                        ]
                        backlog.extendleft(reversed(pr))

                ths.append((400, fin))
                return ths

            for s in (0, 1, 2, 3):
                if s == 2:
                    while not sp23_done[0]:
                        emit_budget(1)
                cells = pass1(0, s)
                backlog.extend(make_pass2(0, s, cells))
            while not kq1_done[0]:
                emit_budget(1)
            for s in (3, 2, 1, 0):
                cells = pass1(1, s)
                backlog.extend(make_pass2(1, s, cells))
            while backlog:
                emit_budget(1)

    nc.compile()
    return nc


_NC = {}


def _get_nc(qk_bias=True):
    if qk_bias not in _NC:
        _NC[qk_bias] = _build(qk_bias=qk_bias)
    return _NC[qk_bias]


def _rope_tables():
    theta = (10000.0 ** (-np.arange(0, DH, 2, dtype=np.float32) / DH)).astype(
        np.float32
    )
    t = np.arange(T, dtype=np.float32)
    sinusoid = np.outer(t, theta).astype(np.float32)  # [T, DH/2]
    sin = np.concatenate([np.sin(sinusoid), np.sin(sinusoid)], axis=1)  # [T, DH]
    cos = np.concatenate([np.cos(sinusoid), np.cos(sinusoid)], axis=1)
    cosT = cos.T  # [DH, T]
    sinT = sin.T
    # sin_perm[e] = sin[(e+32) % 64]
    idx = (np.arange(DH) + 32) % DH
    sinTp = sinT[idx]
    cos2 = np.ascontiguousarray(np.concatenate([cosT, cosT], axis=0))  # [128, T]
    sinp2 = np.ascontiguousarray(np.concatenate([sinTp, sinTp], axis=0))
    return _bf(cos2), _bf(sinp2)


def _perm_matrix():
    p = np.zeros((128, 128), dtype=np.float32)
    for m in range(128):
        blk = m // 64
        k = blk * 64 + (m % 64 + 32) % 64
        p[k, m] = 1.0
    return p


def _mask_matrices():
    # maskT.T @ I adds -400 to S^T[k, q] where k > q (then exp(0.125*s)=0):
    # maskT[a, b] = -400 where b > a
    maskT = -400.0 * np.triu(np.ones((128, 128), dtype=np.float32), 1)
    return maskT, np.eye(128, dtype=np.float32)


def _bf(a):
    return np.ascontiguousarray(np.asarray(a, dtype=np.float32).astype(NPBF16))


def _prepare_in_maps(x, w_qkv, b_qkv, w_proj):
    x = np.asarray(x, dtype=np.float32)
    w_qkv = np.asarray(w_qkv, dtype=np.float32)
    b_qkv = np.asarray(b_qkv, dtype=np.float32)
    w_proj = np.asarray(w_proj, dtype=np.float32)

    cos2, sinp2 = _rope_tables()
    perm = _bf(_perm_matrix())
    maskT, id128 = _mask_matrices()
    maskT, id128 = _bf(maskT), _bf(id128)
    xTs = [_bf(x[b].T) for b in range(B)]

    in_maps = []
    for c in range(N_CORES):
        b, g = divmod(c, 4)
        h0 = g * GH  # first head of the group
        qcols = w_qkv[:, h0 * DH : (h0 + GH) * DH]
        kcols = w_qkv[:, C + h0 * DH : C + (h0 + GH) * DH]
        wqk = _bf(np.concatenate([qcols, kcols], axis=1))
        wv = np.zeros((C, VA), dtype=np.float32)
        bv = np.zeros((1, VA), dtype=np.float32)
        for j in range(GH):
            src = 2 * C + (h0 + j) * DH
            wv[:, j * 65 : j * 65 + DH] = w_qkv[:, src : src + DH]
            bv[0, j * 65 : j * 65 + DH] = b_qkv[src : src + DH]
            bv[0, j * 65 + DH] = 1.0
        bqk = np.concatenate(
            [b_qkv[h0 * DH : (h0 + GH) * DH], b_qkv[C + h0 * DH : C + (h0 + GH) * DH]]
        ).reshape(1, QK_COLS)
        wproj = np.stack(
            [w_proj[(h0 + 2 * p) * DH : (h0 + 2 * p + 2) * DH, :] for p in range(2)]
        )
        in_maps.append(
            {
                "xT": xTs[b],
                "wqk": wqk,
                "wv": _bf(wv),
                "bqk": _bf(bqk),
                "bv": _bf(bv),
                "cosT": cos2,
                "sinTp": sinp2,
                "perm": perm,
                "maskT": maskT,
                "id128": id128,
                "wproj": _bf(wproj),
            }
        )
    return in_maps


def run(x, w_qkv, b_qkv, w_proj, b_proj, trace=False, tmpdir=None):
    qk_bias = bool(np.any(np.asarray(b_qkv, dtype=np.float32)[: 2 * C]))
    nc = _get_nc(qk_bias)
    in_maps = _prepare_in_maps(x, w_qkv, b_qkv, w_proj)
    res = run_bass_kernel_spmd(
        nc, in_maps, list(range(N_CORES)), trace=trace, tmpdir=tmpdir
    )
    b_proj = np.asarray(b_proj, dtype=np.float32)
    out = np.empty((B, T, C), dtype=np.float32)
    for b in range(B):
        acc = res.results[4 * b]["out"].astype(np.float32)
        for g in range(1, 4):
            acc = acc + res.results[4 * b + g]["out"].astype(np.float32)
        out[b] = acc + b_proj
    return out, res


def kernel(x, w_qkv, b_qkv, w_proj, b_proj):
    out, _ = run(x, w_qkv, b_qkv, w_proj, b_proj, trace=False)
    return out


# revision 29
# speedup vs baseline: 1.1840x; 1.0008x over previous
"""Multi-head causal attention with RoPE on 8 Trainium2 NeuronCores.

Reference computation (B=2, T=2048, C=1024, H=16, Dh=64, fp32):
    qkv = x @ w_qkv + b_qkv ; split q,k,v ; RoPE(q), RoPE(k)
    attn = softmax_causal(q k^T / sqrt(Dh)) @ v ; out = attn @ w_proj + b_proj

Sharding: core c = b*4 + g handles batch b and head group g (heads 4g..4g+3).
Data-parallel over batch, tensor-parallel over heads (w_qkv column-split,
w_proj row-split).  Each core emits a partial [T, C] projection output; the
host sums the 4 per-batch partials and adds b_proj.

Per-core kernel (v8).  Heavy matmuls in bf16 (fp32 PSUM accumulation).
Trace-driven design, evolved from a v1 that ran at 333us:

  - Two-phase attention per (head-pair, query-span): pass1 computes the
    S^T k-tiles and immediately exps them into SBUF-resident `et` tiles
    (one fused [128, 2, w] ACT instruction covers both heads of a pair);
    pass2 (PV accumulation + softmax normalization) is deferred into a
    "backlog" of matmul thunks.  Between pass1 steps the emitter pays out
    backlog work (PV of the previous span, V tiles, QKV+RoPE of the other
    pair, output projection), so the ACT-bound exp stream and the PE
    stream overlap without either engine idling for long.
  - The two heads of a pair are emitted back-to-back with K=64
    stationaries at row groups 0/64 (tile_position auto-derived), so
    their S matmuls run concurrently in the two halves of the PE array.
  - softmax: exp on ACT with the 1/sqrt(Dh) scale fused; causal via
    narrowing each k-tile's q-range plus a triangular -400 mask matmul on
    the diagonal 128x128 block.  No max-subtraction (scores are ~N(0,1)
    after scaling; fp32 exp cannot overflow).
  - V is augmented with a ones column so PV also emits the softmax
    denominator.  1/denom: DVE copy of the PSUM denominator row to
    partition 0, vector.reciprocal_approx_fast (the ACT Ln/Exp pair used
    in v1 reloaded the ACT spline tables twice per normalize, 42us), then
    a GPSIMD partition_broadcast, applied by DVE as the PSUM->SBUF move
    of the attention tile.  No fp32r matmuls (their multi-pass mode
    power-throttled the PE clock for the whole attention phase in v1).
  - phase A computes only spans 0-1 of Q/K for pair 0 (all that
    pass1(0,0)/(0,1) need), kc-inner so the PE chases the x DMA; spans
    2-3 are backlog, so attention starts right after the x load lands.
  - pair 1 processes spans in reverse (3,2,1,0) and enqueues its output
    projection right behind each span's normalize, keeping the kernel
    tail short; the projection is emitted per 512-column half so each
    PSUM tile is one bank; output is stored bf16 (summed in fp32 on
    host).
  - Q/K bias matmuls are compiled out when b_qkv is all zeros (the
    build is specialized per the actual inputs; both variants cached).
"""

import numpy as np
import ml_dtypes

from collections import deque

import concourse.bacc as bacc
import concourse.bass as bass
import concourse.mybir as mybir
from concourse.tile import TileContext
from concourse.bass_utils import run_bass_kernel_spmd

F32 = mybir.dt.float32
BF16 = mybir.dt.bfloat16
NPBF16 = np.dtype(ml_dtypes.bfloat16)

B, T, C = 2, 2048, 1024
H, DH = 16, 64
GH = 4  # heads per core
N_CORES = 8
NCHUNK = C // 128  # 8 contraction chunks
NT = T // 128  # 16 token tiles
NSPAN = T // 512  # 4 query spans
QK_COLS = 2 * GH * DH  # 512 = q cols (256) + k cols (256)
VA = GH * (DH + 1)  # 260 = v cols augmented with ones column per head
EXP = mybir.ActivationFunctionType.Exp


def _build(qk_bias=True):
    nc = bacc.Bacc("TRN2", target_bir_lowering=False, debug=False, num_devices=N_CORES)

    xT = nc.dram_tensor("xT", [C, T], BF16, kind="ExternalInput")
    wqk = nc.dram_tensor("wqk", [C, QK_COLS], BF16, kind="ExternalInput")
    wv = nc.dram_tensor("wv", [C, VA], BF16, kind="ExternalInput")
    bqk_d = nc.dram_tensor("bqk", [1, QK_COLS], BF16, kind="ExternalInput")
    bv_d = nc.dram_tensor("bv", [1, VA], BF16, kind="ExternalInput")
    cos_d = nc.dram_tensor("cosT", [128, T], BF16, kind="ExternalInput")
    sinp_d = nc.dram_tensor("sinTp", [128, T], BF16, kind="ExternalInput")
    perm_d = nc.dram_tensor("perm", [128, 128], BF16, kind="ExternalInput")
    maskT_d = nc.dram_tensor("maskT", [128, 128], BF16, kind="ExternalInput")
    id_d = nc.dram_tensor("id128", [128, 128], BF16, kind="ExternalInput")
    wproj_d = nc.dram_tensor("wproj", [2, 128, C], BF16, kind="ExternalInput")
    out_d = nc.dram_tensor("out", [T, C], BF16, kind="ExternalOutput")

    with TileContext(nc) as tc:
        with (
            tc.tile_pool(name="pers", bufs=1) as pers,
            tc.tile_pool(name="ps_s", bufs=2, space="PSUM") as ps_s,
            tc.tile_pool(name="ps_pv", bufs=2, space="PSUM") as ps_pv,
            tc.tile_pool(name="ps_fill", bufs=2, space="PSUM") as ps_fill,
            tc.tile_pool(name="sbw", bufs=1) as sbw,
        ):
            ones = pers.tile([1, 512], BF16, tag="ones")
            nc.vector.memset(ones, 1.0)
            warm = pers.tile([1, 8], F32, tag="warm")
            # Prepay the exp ACT-table load during the DMA ramp.
            nc.scalar.activation(out=warm, in_=ones[0:1, 0:8], func=EXP, scale=0.125)

            # ---------------- input DMA (multi-queue) --------------------
            xt = []
            for kc in range(NCHUNK):
                t = pers.tile([128, T], BF16, tag="xt", bufs=NCHUNK, name=f"xt{kc}")
                eng = nc.sync if kc % 2 == 0 else nc.scalar
                eng.dma_start(out=t, in_=xT[128 * kc : 128 * (kc + 1), :])
                xt.append(t)
            wqk_t = []
            for kc in range(NCHUNK):
                t = pers.tile(
                    [128, QK_COLS], BF16, tag="wqk", bufs=NCHUNK, name=f"wqk{kc}"
                )
                nc.gpsimd.dma_start(out=t, in_=wqk[128 * kc : 128 * (kc + 1), :])
                wqk_t.append(t)
            bqk_sb = pers.tile([1, QK_COLS], BF16, tag="bqk")
            nc.sync.dma_start(out=bqk_sb, in_=bqk_d[:, :])
            bv_sb = pers.tile([1, VA], BF16, tag="bv")
            nc.sync.dma_start(out=bv_sb, in_=bv_d[:, :])
            perm_sb = pers.tile([128, 128], BF16, tag="perm")
            nc.sync.dma_start(out=perm_sb, in_=perm_d[:, :])
            mask_sb = pers.tile([128, 128], BF16, tag="maskT")
            nc.sync.dma_start(out=mask_sb, in_=maskT_d[:, :])
            id_sb = pers.tile([128, 128], BF16, tag="id128")
            nc.sync.dma_start(out=id_sb, in_=id_d[:, :])
            wv_t = []
            for kc in range(NCHUNK):
                t = pers.tile([128, VA], BF16, tag="wv", bufs=NCHUNK, name=f"wv{kc}")
                nc.gpsimd.dma_start(out=t, in_=wv[128 * kc : 128 * (kc + 1), :])
                wv_t.append(t)
            cos_sb = pers.tile([128, T], BF16, tag="cos")
            nc.gpsimd.dma_start(out=cos_sb, in_=cos_d[:, :])
            sinp_sb = pers.tile([128, T], BF16, tag="sinp")
            nc.gpsimd.dma_start(out=sinp_sb, in_=sinp_d[:, :])
            wproj_sb = []
            for p in range(2):
                t = pers.tile([128, C], BF16, tag="wproj", bufs=2, name=f"wproj{p}")
                nc.scalar.dma_start(out=t, in_=wproj_d[p, :, :])
                wproj_sb.append(t)

            # Persistent intermediate tiles
            qkt = []  # 4 tiles [128, T]: Q heads(0,1), Q(2,3), K(0,1), K(2,3)
            for i in range(4):
                t = pers.tile([128, T], BF16, tag="qkt", bufs=4, name=f"qkt{i}")
                qkt.append(t)
            vaug = []  # 16 tiles [128, VA], k-tile-major natural layout V
            for j in range(NT):
                t = pers.tile([128, VA], BF16, tag="vaug", bufs=NT, name=f"vaug{j}")
                vaug.append(t)
            attn = []  # 2 tiles [128, T]: normalized attn^T for head pairs
            for p in range(2):
                t = pers.tile([128, T], BF16, tag="attn", bufs=2, name=f"attn{p}")
                attn.append(t)

            # ---------------- emission helpers ---------------------------
            def rope(ct, sp, pq, permpool, permtag):
                # qkt[ct][:, ss] = pq*cos + perm @ (pq*sin_perm)
                ss = slice(512 * sp, 512 * (sp + 1))
                t2 = sbw.tile([128, 512], BF16, tag="t2", bufs=3, name="t2")
                nc.vector.tensor_mul(t2, pq, sinp_sb[:, ss])
                pp = permpool.tile([128, 512], F32, tag=permtag, name="psperm")
                nc.tensor.matmul(pp, perm_sb, t2, start=True, stop=True)
                nc.vector.tensor_mul(qkt[ct][:, ss], pq, cos_sb[:, ss])
                nc.vector.tensor_add(qkt[ct][:, ss], qkt[ct][:, ss], pp)


            def qk_bias_mm(tile, cs):
                if qk_bias:
                    nc.tensor.matmul(
                        tile, bqk_sb[0:1, cs], ones, start=False, stop=True
                    )

            def qk_part(ct, sps, pool, tag, chunked):
                # Q-or-K column tile for a pair of spans, kc-inner (chasing
                # the x DMA), fused bias + RoPE at the end.
                cs = slice(128 * ct, 128 * (ct + 1))
                if tag == "s":
                    big = pool.tile([128, 1024], F32, tag="s", name="psqkA")
                    tiles = {
                        sp: big[:, 512 * i : 512 * i + 512]
                        for i, sp in enumerate(sps)
                    }
                else:
                    tiles = {
                        sp: pool.tile([128, 512], F32, tag=tag, name="psqk")
                        for sp in sps
                    }
                for kc in range(NCHUNK):
                    for sp in sps:
                        nc.tensor.matmul(
                            tiles[sp],
                            wqk_t[kc][:, cs],
                            xt[kc][:, 512 * sp : 512 * (sp + 1)],
                            start=(kc == 0),
                            stop=(not qk_bias) and kc == NCHUNK - 1,
                        )
                    if chunked:
                        yield
                for sp in sps:
                    qk_bias_mm(tiles[sp], cs)
                    rope(ct, sp, tiles[sp], ps_fill, "ps_fill")
                    if chunked:
                        yield

            def v_tile(it, chunked):
                pv = ps_fill.tile([128, VA], F32, tag="ps_fill", name="psv")
                ts = slice(128 * it, 128 * (it + 1))
                for kc in range(NCHUNK):
                    nc.tensor.matmul(
                        pv, xt[kc][:, ts], wv_t[kc], start=(kc == 0), stop=False
                    )
                    if chunked and kc % 2 == 1 and kc < 7:
                        yield
                nc.tensor.matmul(pv, ones[0:1, 0:128], bv_sb, start=False, stop=True)
                nc.vector.tensor_copy(vaug[it], pv)
                if chunked:
                    yield

            def normalize(pair, idx, s, pv):
                # attn = pv[0:64] * (1/colsum).  The denominator (ones-column
                # PV output, PSUM row 64) is copied to partition 0 — the
                # custom-DVE reciprocal only works at base partition 0 — then
                # broadcast across partitions on GPSIMD.  No ACT tables, no
                # fp32r matmuls (both throttled the v1 kernel).
                po = idx * 64
                ss = slice(512 * s, 512 * (s + 1))
                d0 = sbw.tile([1, 512], F32, tag="d0", bufs=2, name="d0")
                nc.vector.tensor_copy(d0, pv[64:65, :])
                r = sbw.tile([1, 512], F32, tag="r", bufs=2, name="r")
                nc.vector.reciprocal_approx_fast(out=r, in_=d0)
                rbs = sbw.tile([64, 512], F32, tag="rbs", bufs=3, name="rbs")
                nc.gpsimd.partition_broadcast(rbs, r)
                nc.vector.tensor_mul(attn[pair][po : po + 64, ss], pv[0:64, :], rbs)

            def proj_half(it, nh):
                # out[ts, ns] = sum_p attn[p][:, ts]^T @ wproj[p][:, ns]
                ts = slice(128 * it, 128 * (it + 1))
                ns = slice(512 * nh, 512 * (nh + 1))
                ppj = ps_fill.tile([128, 512], F32, tag="ps_fill", name="psproj")
                for p in range(2):
                    nc.tensor.matmul(
                        ppj,
                        attn[p][:, ts],
                        wproj_sb[p][:, ns],
                        start=(p == 0),
                        stop=(p == 1),
                    )
                ob = sbw.tile([128, 512], BF16, tag="ob", bufs=4, name="ob")
                if it < 8:
                    # spans 0-1 of pair 1 are projected after the last exp;
                    # the then-idle ACT takes their PSUM evacuation
                    nc.scalar.copy(ob, ppj)
                else:
                    nc.vector.tensor_copy(ob, ppj)
                eng = nc.sync if (it + nh) % 2 == 0 else nc.scalar
                eng.dma_start(out=out_d[ts, ns], in_=ob)

            # ---------------- phase A: dense PE ramp ---------------------
            # only spans 0-1 of K and Q for pair 0 (all pass1(0,0)/(0,1)
            # needs); spans 2-3 are backlog so attention starts right after
            # the x DMA completes.
            for _ in qk_part(2, (0, 1), ps_s, "s", False):
                pass
            for _ in qk_part(0, (0, 1), ps_s, "s", False):
                pass

            # ------------- two-phase attention with a PE backlog ---------
            # pass1(pair, s): S + fused exp per k-tile, et tiles -> SBUF.
            # pass2(pair, s): PV + normalize, emitted later as backlog
            # thunks between pass1 steps so the PE always has dense,
            # ACT-independent work (the PE only reaches 2.4 GHz after ~3us
            # of continuous execution; any stall resets it to 1.2 GHz).
            backlog = deque()

            def emit_budget(budget):
                while budget > 0 and backlog:
                    cost, fn = backlog.popleft()
                    fn()
                    budget -= cost
                return budget

            def gen_thunks(gen, n, cost):
                return [(cost, (lambda g=gen: next(g, None))) for _ in range(n)]

            kq1_done = [False]
            sp23_done = [False]

            def mark_kq1():
                kq1_done[0] = True

            def mark_sp23():
                sp23_done[0] = True

            backlog.extend(
                gen_thunks(qk_part(2, (2, 3), ps_fill, "ps_fill", True), 10, 1024)
            )
            backlog.extend(
                gen_thunks(qk_part(0, (2, 3), ps_fill, "ps_fill", True), 10, 1024)
            )
            backlog.append((0, mark_sp23))
            backlog.extend(
                th for it in range(NT) for th in gen_thunks(v_tile(it, True), 4, 550)
            )
            for ct in (3, 1):
                for sps in ((0, 1), (2, 3)):
                    backlog.extend(
                        gen_thunks(qk_part(ct, sps, ps_fill, "ps_fill", True), 10, 1024)
                    )
            backlog.append((0, mark_kq1))

            def pass1(pair, s):
                qt, kt = qkt[pair], qkt[2 + pair]
                cells = []
                for j in range(4 * s + 4):
                    st = ps_s.tile([128, 1024], F32, tag="s", name="st")
                    q0 = max(512 * s, 128 * j)
                    w = 512 * (s + 1) - q0
                    diag = s == j // 4
                    for idx in (0, 1):
                        po = idx * 64
                        nc.tensor.matmul(
                            st[:, 512 * idx : 512 * idx + w],
                            kt[po : po + 64, 128 * j : 128 * (j + 1)],
                            qt[po : po + 64, q0 : q0 + w],
                            start=True,
                            stop=not diag,
                        )
                    if diag:
                        for idx in (0, 1):
                            nc.tensor.matmul(
                                st[:, 512 * idx : 512 * idx + 128],
                                mask_sb,
                                id_sb,
                                start=False,
                                stop=True,
                            )
                    et = sbw.tile([128, 1024], BF16, tag="et", bufs=36, name="et")
                    if w == 512:
                        nc.scalar.activation(
                            out=et[:, :], in_=st[:, :], func=EXP, scale=0.125
                        )
                    else:
                        iv = st[:, :].rearrange("p (h c) -> p h c", h=2)[:, :, 0:w]
                        ov = et[:, :].rearrange("p (h c) -> p h c", h=2)[:, :, 0:w]
                        nc.scalar.activation(out=ov, in_=iv, func=EXP, scale=0.125)
                    cells.append((j, q0, w, et))
                    emit_budget(900 + 3 * w)
                return cells

            def make_pass2(pair, s, cells):
                heads = (2 * pair, 2 * pair + 1)
                hold = {}
                ths = []
                for i, (j, q0, w, et) in enumerate(cells):
                    def th(i=i, j=j, q0=q0, w=w, et=et):
                        if i == 0:
                            hold["pv"] = [
                                ps_pv.tile([65, 512], F32, tag="pv", name=f"pspv{k}")
                                for k in (0, 1)
                            ]
                        for idx in (0, 1):
                            h = heads[idx]
                            nc.tensor.matmul(
                                hold["pv"][idx][:, q0 - 512 * s :],
                                vaug[j][:, 65 * h : 65 * (h + 1)],
                                et[:, 512 * idx : 512 * idx + w],
                                start=(j == 0),
                                stop=(j == 4 * s + 3),
                            )
                    ths.append((2 * w, th))

                def fin():
                    for idx in (0, 1):
                        normalize(pair, idx, s, hold["pv"][idx])
                    if pair == 1:
                        # proj right behind the normalize it depends on, so
                        # the PE has work while the normalize chain runs
                        pr = [
                            (1300, (lambda it=it, nh=nh: proj_half(it, nh)))
                            for it in range(4 * s, 4 * s + 4)
                            for nh in range(2)
                        ]
                        backlog.extendleft(reversed(pr))

                ths.append((400, fin))
                return ths

            for s in (0, 1, 2, 3):
                if s == 2:
                    while not sp23_done[0]:
                        emit_budget(1)
                cells = pass1(0, s)
                backlog.extend(make_pass2(0, s, cells))
            while not kq1_done[0]:
                emit_budget(1)
            for s in (3, 2, 1, 0):
                cells = pass1(1, s)
                backlog.extend(make_pass2(1, s, cells))
            while backlog:
                emit_budget(1)

    nc.compile()
    return nc


_NC = {}


def _get_nc(qk_bias=True):
    if qk_bias not in _NC:
        _NC[qk_bias] = _build(qk_bias=qk_bias)
    return _NC[qk_bias]


def _rope_tables():
    theta = (10000.0 ** (-np.arange(0, DH, 2, dtype=np.float32) / DH)).astype(
        np.float32
    )
    t = np.arange(T, dtype=np.float32)
    sinusoid = np.outer(t, theta).astype(np.float32)  # [T, DH/2]
    sin = np.concatenate([np.sin(sinusoid), np.sin(sinusoid)], axis=1)  # [T, DH]
    cos = np.concatenate([np.cos(sinusoid), np.cos(sinusoid)], axis=1)
    cosT = cos.T  # [DH, T]
    sinT = sin.T
    # sin_perm[e] = sin[(e+32) % 64]
    idx = (np.arange(DH) + 32) % DH
    sinTp = sinT[idx]
    cos2 = np.ascontiguousarray(np.concatenate([cosT, cosT], axis=0))  # [128, T]
    sinp2 = np.ascontiguousarray(np.concatenate([sinTp, sinTp], axis=0))
    return _bf(cos2), _bf(sinp2)


def _perm_matrix():
    p = np.zeros((128, 128), dtype=np.float32)
    for m in range(128):
        blk = m // 64
        k = blk * 64 + (m % 64 + 32) % 64
        p[k, m] = 1.0
    return p


def _mask_matrices():
    # maskT.T @ I adds -400 to S^T[k, q] where k > q (then exp(0.125*s)=0):
    # maskT[a, b] = -400 where b > a
    maskT = -400.0 * np.triu(np.ones((128, 128), dtype=np.float32), 1)
    return maskT, np.eye(128, dtype=np.float32)


def _bf(a):
    return np.ascontiguousarray(np.asarray(a, dtype=np.float32).astype(NPBF16))


def _prepare_in_maps(x, w_qkv, b_qkv, w_proj):
    x = np.asarray(x, dtype=np.float32)
    w_qkv = np.asarray(w_qkv, dtype=np.float32)
    b_qkv = np.asarray(b_qkv, dtype=np.float32)
    w_proj = np.asarray(w_proj, dtype=np.float32)

    cos2, sinp2 = _rope_tables()
    perm = _bf(_perm_matrix())
    maskT, id128 = _mask_matrices()
    maskT, id128 = _bf(maskT), _bf(id128)
    xTs = [_bf(x[b].T) for b in range(B)]

    in_maps = []
    for c in range(N_CORES):
        b, g = divmod(c, 4)
        h0 = g * GH  # first head of the group
        qcols = w_qkv[:, h0 * DH : (h0 + GH) * DH]
        kcols = w_qkv[:, C + h0 * DH : C + (h0 + GH) * DH]
        wqk = _bf(np.concatenate([qcols, kcols], axis=1))
        wv = np.zeros((C, VA), dtype=np.float32)
        bv = np.zeros((1, VA), dtype=np.float32)
        for j in range(GH):
            src = 2 * C + (h0 + j) * DH
            wv[:, j * 65 : j * 65 + DH] = w_qkv[:, src : src + DH]
            bv[0, j * 65 : j * 65 + DH] = b_qkv[src : src + DH]
            bv[0, j * 65 + DH] = 1.0
        bqk = np.concatenate(
            [b_qkv[h0 * DH : (h0 + GH) * DH], b_qkv[C + h0 * DH : C + (h0 + GH) * DH]]
        ).reshape(1, QK_COLS)
        wproj = np.stack(
            [w_proj[(h0 + 2 * p) * DH : (h0 + 2 * p + 2) * DH, :] for p in range(2)]
        )
        in_maps.append(
            {
                "xT": xTs[b],
                "wqk": wqk,
                "wv": _bf(wv),
                "bqk": _bf(bqk),
                "bv": _bf(bv),
                "cosT": cos2,
                "sinTp": sinp2,
                "perm": perm,
                "maskT": maskT,
                "id128": id128,
                "wproj": _bf(wproj),
            }
        )
    return in_maps


def run(x, w_qkv, b_qkv, w_proj, b_proj, trace=False, tmpdir=None):
    qk_bias = bool(np.any(np.asarray(b_qkv, dtype=np.float32)[: 2 * C]))
    nc = _get_nc(qk_bias)
    in_maps = _prepare_in_maps(x, w_qkv, b_qkv, w_proj)
    res = run_bass_kernel_spmd(
        nc, in_maps, list(range(N_CORES)), trace=trace, tmpdir=tmpdir
    )
    b_proj = np.asarray(b_proj, dtype=np.float32)
    out = np.empty((B, T, C), dtype=np.float32)
    for b in range(B):
        acc = res.results[4 * b]["out"].astype(np.float32)
        for g in range(1, 4):
            acc = acc + res.results[4 * b + g]["out"].astype(np.float32)
        out[b] = acc + b_proj
    return out, res


def kernel(x, w_qkv, b_qkv, w_proj, b_proj):
    out, _ = run(x, w_qkv, b_qkv, w_proj, b_proj, trace=False)
    return out


# revision 30
# speedup vs baseline: 1.2050x; 1.0178x over previous
"""Multi-head causal attention with RoPE on 8 Trainium2 NeuronCores.

Reference computation (B=2, T=2048, C=1024, H=16, Dh=64, fp32):
    qkv = x @ w_qkv + b_qkv ; split q,k,v ; RoPE(q), RoPE(k)
    attn = softmax_causal(q k^T / sqrt(Dh)) @ v ; out = attn @ w_proj + b_proj

Sharding: core c = b*4 + g handles batch b and head group g (heads 4g..4g+3).
Data-parallel over batch, tensor-parallel over heads (w_qkv column-split,
w_proj row-split).  Each core emits a partial [T, C] projection output; the
host sums the 4 per-batch partials and adds b_proj.

Per-core kernel, v2.  Heavy matmuls in bf16 (fp32 PSUM accumulation).
Design is driven by two trace findings on the v1 kernel: (a) the scalar
engine is the serial bottleneck of the attention phase (exp of all causal
scores at 1 elem/cycle/lane), and (b) the PE HAM clock gate throttled the
PE to 1.2 GHz for the whole attention phase (fp32r broadcast matmuls +
ACT table reloads for the Ln/Exp reciprocal created PE idle gaps).

  - The program is emitted as one interleaved stream: while the ACT-bound
    attention chunk pipeline of head pair 0 runs, the PE executes "filler"
    matmuls (V projection tiles, QKV+RoPE for pair 1) from the same queue;
    during pair 1's attention the fillers are the output-projection tiles
    of already-normalized spans.  The PE never idles long enough to drop
    to the throttled clock.
  - S^T tile = K_j Q^T per head; the two heads of a pair are emitted
    back-to-back with K=64 stationaries at row groups 0/64 (tile_position
    auto-derived from base partitions), so they run concurrently in the
    two halves of the PE array.
  - softmax: exp on ACT with the 1/sqrt(Dh) scale fused; causal via
    narrowing each k-tile's q-range plus one triangular -400 mask matmul
    on the diagonal 128x128 block.
  - V is augmented with a ones column so PV also emits the softmax
    denominator; 1/denom via vector.reciprocal_approx_fast (no ACT table
    switch), broadcast across partitions with a stride-0-source DMA, and
    applied by DVE as the PSUM->SBUF move of the attention tile.
  - projection: per head-pair stationary attn^T tiles vs w_proj rows,
    emitted per 512-column half so each PSUM tile is one bank.
"""

import numpy as np
import ml_dtypes

from collections import deque

import concourse.bacc as bacc
import concourse.bass as bass
import concourse.mybir as mybir
from concourse.tile import TileContext
from concourse.bass_utils import run_bass_kernel_spmd

F32 = mybir.dt.float32
BF16 = mybir.dt.bfloat16
NPBF16 = np.dtype(ml_dtypes.bfloat16)

B, T, C = 2, 2048, 1024
H, DH = 16, 64
GH = 4  # heads per core
N_CORES = 8
NCHUNK = C // 128  # 8 contraction chunks
NT = T // 128  # 16 token tiles
NSPAN = T // 512  # 4 query spans
QK_COLS = 2 * GH * DH  # 512 = q cols (256) + k cols (256)
VA = GH * (DH + 1)  # 260 = v cols augmented with ones column per head
EXP = mybir.ActivationFunctionType.Exp


def _build(qk_bias=True):
    nc = bacc.Bacc("TRN2", target_bir_lowering=False, debug=False, num_devices=N_CORES)

    xT = nc.dram_tensor("xT", [C, T], BF16, kind="ExternalInput")
    wqk = nc.dram_tensor("wqk", [C, QK_COLS], BF16, kind="ExternalInput")
    wv = nc.dram_tensor("wv", [C, VA], BF16, kind="ExternalInput")
    bqk_d = nc.dram_tensor("bqk", [1, QK_COLS], BF16, kind="ExternalInput")
    bv_d = nc.dram_tensor("bv", [1, VA], BF16, kind="ExternalInput")
    cos_d = nc.dram_tensor("cosT", [128, T], BF16, kind="ExternalInput")
    sinp_d = nc.dram_tensor("sinTp", [128, T], BF16, kind="ExternalInput")
    perm_d = nc.dram_tensor("perm", [128, 128], BF16, kind="ExternalInput")
    wproj_d = nc.dram_tensor("wproj", [2, 128, C], BF16, kind="ExternalInput")
    out_d = nc.dram_tensor("out", [T, C], BF16, kind="ExternalOutput")

    with TileContext(nc) as tc:
        with (
            tc.tile_pool(name="pers", bufs=1) as pers,
            tc.tile_pool(name="ps_s", bufs=2, space="PSUM") as ps_s,
            tc.tile_pool(name="ps_pv", bufs=2, space="PSUM") as ps_pv,
            tc.tile_pool(name="ps_fill", bufs=2, space="PSUM") as ps_fill,
            tc.tile_pool(name="sbw", bufs=1) as sbw,
        ):
            ones = pers.tile([1, 512], BF16, tag="ones")
            nc.vector.memset(ones, 1.0)
            warm = pers.tile([1, 8], F32, tag="warm")
            # Prepay the exp ACT-table load during the DMA ramp.
            nc.scalar.activation(out=warm, in_=ones[0:1, 0:8], func=EXP, scale=0.125)

            # ---------------- input DMA (multi-queue) --------------------
            xt = []
            for kc in range(NCHUNK):
                t = pers.tile([128, T], BF16, tag="xt", bufs=NCHUNK, name=f"xt{kc}")
                eng = nc.sync if kc % 2 == 0 else nc.scalar
                eng.dma_start(out=t, in_=xT[128 * kc : 128 * (kc + 1), :])
                xt.append(t)
            wqk_t = []
            for kc in range(NCHUNK):
                t = pers.tile(
                    [128, QK_COLS], BF16, tag="wqk", bufs=NCHUNK, name=f"wqk{kc}"
                )
                nc.gpsimd.dma_start(out=t, in_=wqk[128 * kc : 128 * (kc + 1), :])
                wqk_t.append(t)
            bqk_sb = pers.tile([1, QK_COLS], BF16, tag="bqk")
            nc.sync.dma_start(out=bqk_sb, in_=bqk_d[:, :])
            bv_sb = pers.tile([1, VA], BF16, tag="bv")
            nc.sync.dma_start(out=bv_sb, in_=bv_d[:, :])
            perm_sb = pers.tile([128, 128], BF16, tag="perm")
            nc.sync.dma_start(out=perm_sb, in_=perm_d[:, :])
            wv_t = []
            for kc in range(NCHUNK):
                t = pers.tile([128, VA], BF16, tag="wv", bufs=NCHUNK, name=f"wv{kc}")
                nc.gpsimd.dma_start(out=t, in_=wv[128 * kc : 128 * (kc + 1), :])
                wv_t.append(t)
            cos_sb = pers.tile([128, T], BF16, tag="cos")
            nc.gpsimd.dma_start(out=cos_sb, in_=cos_d[:, :])
            sinp_sb = pers.tile([128, T], BF16, tag="sinp")
            nc.gpsimd.dma_start(out=sinp_sb, in_=sinp_d[:, :])
            wproj_sb = []
            for p in range(2):
                t = pers.tile([128, C], BF16, tag="wproj", bufs=2, name=f"wproj{p}")
                nc.scalar.dma_start(out=t, in_=wproj_d[p, :, :])
                wproj_sb.append(t)

            # Persistent intermediate tiles
            qkt = []  # 4 tiles [128, T]: Q heads(0,1), Q(2,3), K(0,1), K(2,3)
            for i in range(4):
                t = pers.tile([128, T], BF16, tag="qkt", bufs=4, name=f"qkt{i}")
                qkt.append(t)
            vaug = []  # 16 tiles [128, VA], k-tile-major natural layout V
            for j in range(NT):
                t = pers.tile([128, VA], BF16, tag="vaug", bufs=NT, name=f"vaug{j}")
                vaug.append(t)
            attn = []  # 2 tiles [128, T]: normalized attn^T for head pairs
            for p in range(2):
                t = pers.tile([128, T], BF16, tag="attn", bufs=2, name=f"attn{p}")
                attn.append(t)

            # ---------------- emission helpers ---------------------------
            def rope(ct, sp, pq, permpool, permtag):
                # qkt[ct][:, ss] = pq*cos + perm @ (pq*sin_perm)
                ss = slice(512 * sp, 512 * (sp + 1))
                t2 = sbw.tile([128, 512], BF16, tag="t2", bufs=3, name="t2")
                nc.vector.tensor_mul(t2, pq, sinp_sb[:, ss])
                pp = permpool.tile([128, 512], F32, tag=permtag, name="psperm")
                nc.tensor.matmul(pp, perm_sb, t2, start=True, stop=True)
                nc.vector.tensor_mul(qkt[ct][:, ss], pq, cos_sb[:, ss])
                nc.vector.tensor_add(qkt[ct][:, ss], qkt[ct][:, ss], pp)


            def qk_bias_mm(tile, cs):
                if qk_bias:
                    nc.tensor.matmul(
                        tile, bqk_sb[0:1, cs], ones, start=False, stop=True
                    )

            def qk_part(ct, sps, pool, tag, chunked):
                # Q-or-K column tile for a pair of spans, kc-inner (chasing
                # the x DMA), fused bias + RoPE at the end.
                cs = slice(128 * ct, 128 * (ct + 1))
                if tag == "s":
                    big = pool.tile([128, 1024], F32, tag="s", name="psqkA")
                    tiles = {
                        sp: big[:, 512 * i : 512 * i + 512]
                        for i, sp in enumerate(sps)
                    }
                else:
                    tiles = {
                        sp: pool.tile([128, 512], F32, tag=tag, name="psqk")
                        for sp in sps
                    }
                for kc in range(NCHUNK):
                    for sp in sps:
                        nc.tensor.matmul(
                            tiles[sp],
                            wqk_t[kc][:, cs],
                            xt[kc][:, 512 * sp : 512 * (sp + 1)],
                            start=(kc == 0),
                            stop=(not qk_bias) and kc == NCHUNK - 1,
                        )
                    if chunked:
                        yield
                for sp in sps:
                    qk_bias_mm(tiles[sp], cs)
                    rope(ct, sp, tiles[sp], ps_fill, "ps_fill")
                    if chunked:
                        yield

            def v_tile(it, chunked):
                pv = ps_fill.tile([128, VA], F32, tag="ps_fill", name="psv")
                ts = slice(128 * it, 128 * (it + 1))
                for kc in range(NCHUNK):
                    nc.tensor.matmul(
                        pv, xt[kc][:, ts], wv_t[kc], start=(kc == 0), stop=False
                    )
                    if chunked and kc % 2 == 1 and kc < 7:
                        yield
                nc.tensor.matmul(pv, ones[0:1, 0:128], bv_sb, start=False, stop=True)
                nc.vector.tensor_copy(vaug[it], pv)
                if chunked:
                    yield

            def normalize(pair, idx, s, pv):
                # attn = pv[0:64] * (1/colsum).  The denominator (ones-column
                # PV output, PSUM row 64) is copied to partition 0 — the
                # custom-DVE reciprocal only works at base partition 0 — then
                # broadcast across partitions on GPSIMD.  No ACT tables, no
                # fp32r matmuls (both throttled the v1 kernel).
                po = idx * 64
                ss = slice(512 * s, 512 * (s + 1))
                d0 = sbw.tile([1, 512], F32, tag="d0", bufs=2, name="d0")
                nc.vector.tensor_copy(d0, pv[64:65, :])
                r = sbw.tile([1, 512], F32, tag="r", bufs=2, name="r")
                nc.vector.reciprocal_approx_fast(out=r, in_=d0)
                rbs = sbw.tile([64, 512], F32, tag="rbs", bufs=3, name="rbs")
                nc.gpsimd.partition_broadcast(rbs, r)
                nc.vector.tensor_mul(attn[pair][po : po + 64, ss], pv[0:64, :], rbs)

            def proj_half(it, nh):
                # out[ts, ns] = sum_p attn[p][:, ts]^T @ wproj[p][:, ns]
                ts = slice(128 * it, 128 * (it + 1))
                ns = slice(512 * nh, 512 * (nh + 1))
                ppj = ps_fill.tile([128, 512], F32, tag="ps_fill", name="psproj")
                for p in range(2):
                    nc.tensor.matmul(
                        ppj,
                        attn[p][:, ts],
                        wproj_sb[p][:, ns],
                        start=(p == 0),
                        stop=(p == 1),
                    )
                ob = sbw.tile([128, 512], BF16, tag="ob", bufs=4, name="ob")
                if it < 8:
                    # spans 0-1 of pair 1 are projected after the last exp;
                    # the then-idle ACT takes their PSUM evacuation
                    nc.scalar.copy(ob, ppj)
                else:
                    nc.vector.tensor_copy(ob, ppj)
                eng = nc.sync if (it + nh) % 2 == 0 else nc.scalar
                eng.dma_start(out=out_d[ts, ns], in_=ob)

            # ---------------- phase A: dense PE ramp ---------------------
            # only spans 0-1 of K and Q for pair 0 (all pass1(0,0)/(0,1)
            # needs); spans 2-3 are backlog so attention starts right after
            # the x DMA completes.
            for _ in qk_part(2, (0, 1), ps_s, "s", False):
                pass
            for _ in qk_part(0, (0, 1), ps_s, "s", False):
                pass

            # ------------- two-phase attention with a PE backlog ---------
            # pass1(pair, s): S + fused exp per k-tile, et tiles -> SBUF.
            # pass2(pair, s): PV + normalize, emitted later as backlog
            # thunks between pass1 steps so the PE always has dense,
            # ACT-independent work (the PE only reaches 2.4 GHz after ~3us
            # of continuous execution; any stall resets it to 1.2 GHz).
            backlog = deque()

            def emit_budget(budget):
                while budget > 0 and backlog:
                    cost, fn = backlog.popleft()
                    fn()
                    budget -= cost
                return budget

            def gen_thunks(gen, n, cost):
                return [(cost, (lambda g=gen: next(g, None))) for _ in range(n)]

            kq1_done = [False]
            sp23_done = [False]

            def mark_kq1():
                kq1_done[0] = True

            def mark_sp23():
                sp23_done[0] = True

            backlog.extend(
                gen_thunks(qk_part(2, (2, 3), ps_fill, "ps_fill", True), 10, 1024)
            )
            backlog.extend(
                gen_thunks(qk_part(0, (2, 3), ps_fill, "ps_fill", True), 10, 1024)
            )
            backlog.append((0, mark_sp23))
            backlog.extend(
                th for it in range(NT) for th in gen_thunks(v_tile(it, True), 4, 550)
            )
            for ct in (3, 1):
                for sps in ((0, 1), (2, 3)):
                    backlog.extend(
                        gen_thunks(qk_part(ct, sps, ps_fill, "ps_fill", True), 10, 1024)
                    )
            backlog.append((0, mark_kq1))

            def pass1(pair, s):
                qt, kt = qkt[pair], qkt[2 + pair]
                cells = []
                for j in range(4 * s + 4):
                    st = ps_s.tile([128, 1024], F32, tag="s", name="st")
                    q0 = max(512 * s, 128 * j)
                    w = 512 * (s + 1) - q0
                    diag = s == j // 4
                    for idx in (0, 1):
                        po = idx * 64
                        nc.tensor.matmul(
                            st[:, 512 * idx : 512 * idx + w],
                            kt[po : po + 64, 128 * j : 128 * (j + 1)],
                            qt[po : po + 64, q0 : q0 + w],
                            start=True,
                            stop=True,
                        )
                    et = sbw.tile([128, 1024], BF16, tag="et", bufs=36, name="et")
                    if w == 512:
                        nc.scalar.activation(
                            out=et[:, :], in_=st[:, :], func=EXP, scale=0.125
                        )
                    else:
                        iv = st[:, :].rearrange("p (h c) -> p h c", h=2)[:, :, 0:w]
                        ov = et[:, :].rearrange("p (h c) -> p h c", h=2)[:, :, 0:w]
                        nc.scalar.activation(out=ov, in_=iv, func=EXP, scale=0.125)
                    if diag:
                        # causal: zero et where key k > query (q0 + c); the
                        # triangle lives in the first 128 columns (one DVE +
                        # one GPSIMD affine_select instead of PE mask matmuls)
                        tw = min(w, 128)
                        for idx, eng in ((0, nc.gpsimd), (1, nc.gpsimd)):
                            sl = et[:, 512 * idx : 512 * idx + tw]
                            eng.affine_select(
                                out=sl,
                                in_=sl,
                                compare_op=mybir.AluOpType.is_ge,
                                fill=0.0,
                                base=0,
                                pattern=[[1, tw]],
                                channel_multiplier=-1,
                            )
                    cells.append((j, q0, w, et))
                    emit_budget(900 + 3 * w)
                return cells

            def make_pass2(pair, s, cells):
                heads = (2 * pair, 2 * pair + 1)
                hold = {}
                ths = []
                for i, (j, q0, w, et) in enumerate(cells):
                    def th(i=i, j=j, q0=q0, w=w, et=et):
                        if i == 0:
                            hold["pv"] = [
                                ps_pv.tile([65, 512], F32, tag="pv", name=f"pspv{k}")
                                for k in (0, 1)
                            ]
                        for idx in (0, 1):
                            h = heads[idx]
                            nc.tensor.matmul(
                                hold["pv"][idx][:, q0 - 512 * s :],
                                vaug[j][:, 65 * h : 65 * (h + 1)],
                                et[:, 512 * idx : 512 * idx + w],
                                start=(j == 0),
                                stop=(j == 4 * s + 3),
                            )
                    ths.append((2 * w, th))

                def fin():
                    for idx in (0, 1):
                        normalize(pair, idx, s, hold["pv"][idx])
                    if pair == 1:
                        # proj right behind the normalize it depends on, so
                        # the PE has work while the normalize chain runs
                        pr = [
                            (1300, (lambda it=it, nh=nh: proj_half(it, nh)))
                            for it in range(4 * s, 4 * s + 4)
                            for nh in range(2)
                        ]
                        backlog.extendleft(reversed(pr))

                ths.append((400, fin))
                return ths

            for s in (0, 1, 2, 3):
                if s == 2:
                    while not sp23_done[0]:
                        emit_budget(1)
                cells = pass1(0, s)
                backlog.extend(make_pass2(0, s, cells))
            while not kq1_done[0]:
                emit_budget(1)
            for s in (3, 2, 1, 0):
                cells = pass1(1, s)
                backlog.extend(make_pass2(1, s, cells))
            while backlog:
                emit_budget(1)

    nc.compile()
    return nc


_NC = {}


def _get_nc(qk_bias=True):
    if qk_bias not in _NC:
        _NC[qk_bias] = _build(qk_bias=qk_bias)
    return _NC[qk_bias]


def _rope_tables():
    theta = (10000.0 ** (-np.arange(0, DH, 2, dtype=np.float32) / DH)).astype(
        np.float32
    )
    t = np.arange(T, dtype=np.float32)
    sinusoid = np.outer(t, theta).astype(np.float32)  # [T, DH/2]
    sin = np.concatenate([np.sin(sinusoid), np.sin(sinusoid)], axis=1)  # [T, DH]
    cos = np.concatenate([np.cos(sinusoid), np.cos(sinusoid)], axis=1)
    cosT = cos.T  # [DH, T]
    sinT = sin.T
    # sin_perm[e] = sin[(e+32) % 64]
    idx = (np.arange(DH) + 32) % DH
    sinTp = sinT[idx]
    cos2 = np.ascontiguousarray(np.concatenate([cosT, cosT], axis=0))  # [128, T]
    sinp2 = np.ascontiguousarray(np.concatenate([sinTp, sinTp], axis=0))
    return _bf(cos2), _bf(sinp2)


def _perm_matrix():
    p = np.zeros((128, 128), dtype=np.float32)
    for m in range(128):
        blk = m // 64
        k = blk * 64 + (m % 64 + 32) % 64
        p[k, m] = 1.0
    return p


def _bf(a):
    return np.ascontiguousarray(np.asarray(a, dtype=np.float32).astype(NPBF16))


def _prepare_in_maps(x, w_qkv, b_qkv, w_proj):
    x = np.asarray(x, dtype=np.float32)
    w_qkv = np.asarray(w_qkv, dtype=np.float32)
    b_qkv = np.asarray(b_qkv, dtype=np.float32)
    w_proj = np.asarray(w_proj, dtype=np.float32)

    cos2, sinp2 = _rope_tables()
    perm = _bf(_perm_matrix())
    xTs = [_bf(x[b].T) for b in range(B)]

    in_maps = []
    for c in range(N_CORES):
        b, g = divmod(c, 4)
        h0 = g * GH  # first head of the group
        qcols = w_qkv[:, h0 * DH : (h0 + GH) * DH]
        kcols = w_qkv[:, C + h0 * DH : C + (h0 + GH) * DH]
        wqk = _bf(np.concatenate([qcols, kcols], axis=1))
        wv = np.zeros((C, VA), dtype=np.float32)
        bv = np.zeros((1, VA), dtype=np.float32)
        for j in range(GH):
            src = 2 * C + (h0 + j) * DH
            wv[:, j * 65 : j * 65 + DH] = w_qkv[:, src : src + DH]
            bv[0, j * 65 : j * 65 + DH] = b_qkv[src : src + DH]
            bv[0, j * 65 + DH] = 1.0
        bqk = np.concatenate(
            [b_qkv[h0 * DH : (h0 + GH) * DH], b_qkv[C + h0 * DH : C + (h0 + GH) * DH]]
        ).reshape(1, QK_COLS)
        wproj = np.stack(
            [w_proj[(h0 + 2 * p) * DH : (h0 + 2 * p + 2) * DH, :] for p in range(2)]
        )
        in_maps.append(
            {
                "xT": xTs[b],
                "wqk": wqk,
                "wv": _bf(wv),
                "bqk": _bf(bqk),
                "bv": _bf(bv),
                "cosT": cos2,
                "sinTp": sinp2,
                "perm": perm,
                "wproj": _bf(wproj),
            }
        )
    return in_maps


def run(x, w_qkv, b_qkv, w_proj, b_proj, trace=False, tmpdir=None):
    qk_bias = bool(np.any(np.asarray(b_qkv, dtype=np.float32)[: 2 * C]))
    nc = _get_nc(qk_bias)
    in_maps = _prepare_in_maps(x, w_qkv, b_qkv, w_proj)
    res = run_bass_kernel_spmd(
        nc, in_maps, list(range(N_CORES)), trace=trace, tmpdir=tmpdir
    )
    b_proj = np.asarray(b_proj, dtype=np.float32)
    out = np.empty((B, T, C), dtype=np.float32)
    for b in range(B):
        acc = res.results[4 * b]["out"].astype(np.float32)
        for g in range(1, 4):
            acc = acc + res.results[4 * b + g]["out"].astype(np.float32)
        out[b] = acc + b_proj
    return out, res


def kernel(x, w_qkv, b_qkv, w_proj, b_proj):
    out, _ = run(x, w_qkv, b_qkv, w_proj, b_proj, trace=False)
    return out
